# revision 1
# baseline (speedup 1.0000x reference)
"""Trainium2 Bass kernel for the FlowNet-style correlation module.

out[b, u*21+v, i, j] = sum_c x1[b,c,i,j] * x2pad[b,c,i+u,j+v]
with x1, x2: [4, 128, 128, 128] fp32, pad=10, window 21x21 (441 output channels).

Strategy
--------
Sharding: 8 cores = (batch 4) x (H halves). Each core handles one batch's
64-row slab: x1 slice [C=128, 64, 128] and a host-prepadded x2 slice
[C=128, 84, 148] (rows/cols include the +-10 zero halo).

Per core the correlation is computed as blocked Gram matmuls on the tensor
engine using PE column-tiling: each 4x8 pixel block of x1 (M=32) is a
stationary operand on one 32-column group of the PE array
(tile_position=(0,32g)), and four such blocks run CONCURRENTLY against their
own 24x28 x2pad halo windows (N=672, split into two 336-column PSUM passes).
Hardware-verified: 4 concurrent M=32 col-tiles stream at the same wall time
as a single M=128 matmul, so the small-block shape costs no PE time while
cutting the shipped-Gram inflation from 2.29x (8x16 blocks) to 1.52x.

Inputs are split on the host into fp16 hi + lo parts and each Gram tile is
accumulated as h1.h2 + h1.l2 + l1.h2 - three full-rate fp16 matmuls whose
products are exact in the fp32 PSUM accumulator - giving fp32-level accuracy
(measured 2.9e-07 scale-relative) at 3 cycles/column.

Each output pixel's 21x21 window is a per-partition band of its Gram tile; a
per-partition-offset band cannot be expressed by any on-chip access pattern
(and DMA has no PSUM route), so the device ships the full Gram tiles and the
host extracts the band while unsharding. The kernel is DMA-bound: ~22MB Gram
out (16 batched 1.38MB DMAs, above the ~1MB efficiency knee) + ~10.6MB in
per core at ~360GB/s -> ~94us/core estimated.
"""

import numpy as np

import concourse.mybir as mybir
import concourse.tile as tile
from concourse import bacc
from concourse.bass_utils import run_bass_kernel_spmd

# Problem constants (hardcoded; kernel.py must be self-contained).
B, C, H, W = 4, 128, 128, 128
PAD = 10
WIN = 21  # correlation window side; WIN**2 = 441 output channels
N_CORES = 8
ROWS = H // 2  # 64 output rows per core
HROWS = ROWS + 2 * PAD  # 84 x2pad rows per core
PW = W + 2 * PAD  # 148 x2pad cols

# Pixel blocking: M-block = DI x DJ = 32 pixels on one PE column group;
# 4 blocks (one quad) run concurrently on the 4 column groups.
DI, DJ = 4, 8
NR, NS = DI + WIN - 1, DJ + WIN - 1  # 24, 28
NBI, NBJ = ROWS // DI, W // DJ  # 16, 16
NQJ = NBJ // 4  # 4 quads per block-row
NQUAD = NBI * NQJ  # 64 quads per core
NFREE = NR * NS  # 672 Gram columns per block
RSPLIT = NR // 2  # 12 rows -> 336 columns per matmul (PSUM bank holds 512 fp32)
NCOL = RSPLIT * NS  # 336

F32 = mybir.dt.float32
F16 = mybir.dt.float16

_NC_CACHE = {}

# Tunables (overridable via _build_nc kwargs for experiments).
GRAM_BUFS = 6
PSUM_BUFS = 8
DVE_COLS = 240  # columns of each 336-col PSUM tile copied by DVE (rest: ACT)
BI_GROUPS = [(0, 2), (2, 6), (6, 11), (11, 16)]


QBATCH = 4  # quads per output DMA (1.38MB transfers, above the ~1MB DMA knee)
# Per-DMA quad counts (must sum to 64). Uniform 4-quad batches measured best:
# head/tail-trimmed schedules pay more in extra per-DMA fixed cost than the
# shorter pipeline fill/drain saves.
QSCHED = [4] * 16


def _qsched(qbatch):
    if qbatch is None:
        return list(QSCHED)
    return [qbatch] * (NQUAD // qbatch)


def _build_nc(
    gram_bufs=None, psum_bufs=None, dve_cols=None, bi_groups=None,
    qbatch=None, passes=3, alt_dge=False,
):
    gram_bufs = GRAM_BUFS if gram_bufs is None else gram_bufs
    psum_bufs = PSUM_BUFS if psum_bufs is None else psum_bufs
    dve_cols = DVE_COLS if dve_cols is None else dve_cols
    bi_groups = BI_GROUPS if bi_groups is None else bi_groups
    qsched = _qsched(qbatch)
    assert sum(qsched) == NQUAD
    key = (gram_bufs, psum_bufs, dve_cols, tuple(bi_groups), tuple(qsched), passes, alt_dge)
    if key in _NC_CACHE:
        return _NC_CACHE[key]
    nc = bacc.Bacc("TRN2", target_bir_lowering=False, debug=False, num_devices=N_CORES)
    # x1 arrives host-rearranged so each 4x8 block's 32 pixels are contiguous
    # (the matmul stationary operand AP must have a single free dimension).
    # h/l stay as 4 separate tensors: packing them into one tensor was tried
    # and measured worse (the combined first-chunk DMA delays the h-only
    # first matmul pass by ~3us).
    NBLK = NBI * NBJ
    x1hd = nc.dram_tensor("x1h", [C, NBLK, DI * DJ], F16, kind="ExternalInput")
    x1ld = nc.dram_tensor("x1l", [C, NBLK, DI * DJ], F16, kind="ExternalInput")
    x2hd = nc.dram_tensor("x2h", [C, HROWS, PW], F16, kind="ExternalInput")
    x2ld = nc.dram_tensor("x2l", [C, HROWS, PW], F16, kind="ExternalInput")
    # Flat [partition, quad-major columns] layout: quad q's Gram tile lives at
    # columns [q*2*NCOL, (q+1)*2*NCOL) regardless of the DMA batch schedule.
    gout = nc.dram_tensor(
        "gout", [128, NQUAD * 2 * NCOL], F32, kind="ExternalOutput"
    )

    with tile.TileContext(nc) as tc:
        with (
            tc.tile_pool(name="inp", bufs=1) as inp,
            tc.tile_pool(name="gram", bufs=gram_bufs) as gp,
            tc.tile_pool(name="psum", bufs=psum_bufs, space="PSUM") as pp,
        ):
            x1ht = inp.tile([C, NBLK, DI * DJ], F16)
            x1lt = inp.tile([C, NBLK, DI * DJ], F16)
            x2ht = inp.tile([C, HROWS, PW], F16)
            x2lt = inp.tile([C, HROWS, PW], F16)
            # Chunked input loads (x1 blocks + the x2 rows they need first,
            # h parts before l so pass-1 matmuls start earliest).
            rprev = 0
            for glo, ghi in bi_groups:
                blo, bhi = glo * NBJ, ghi * NBJ
                rhi = min(HROWS, (ghi - 1) * DI + NR)
                nc.sync.dma_start(x1ht[:, blo:bhi, :], x1hd[:, blo:bhi, :])
                nc.sync.dma_start(x2ht[:, rprev:rhi, :], x2hd[:, rprev:rhi, :])
                nc.sync.dma_start(x1lt[:, blo:bhi, :], x1ld[:, blo:bhi, :])
                nc.sync.dma_start(x2lt[:, rprev:rhi, :], x2ld[:, rprev:rhi, :])
                rprev = rhi

            # Map quad index -> (batch start quad, batch size)
            qstart = {}
            q0 = 0
            for qb in qsched:
                for q in range(q0, q0 + qb):
                    qstart[q] = (q0, qb)
                q0 += qb
            g = None
            for bi in range(NBI):
                i0 = bi * DI
                for qj in range(NQJ):
                    quad = bi * NQJ + qj
                    b0, qb = qstart[quad]
                    if quad == b0:
                        g = gp.tile([128, qb * 2 * NCOL], F32, tag="g")
                    qoff = (quad - b0) * 2 * NCOL
                    for h in range(2):
                        ps = pp.tile([128, NCOL], F32, tag="ps")
                        r0 = i0 + h * RSPLIT
                        for grp in range(4):
                            blk = bi * NBJ + qj * 4 + grp
                            j0 = (qj * 4 + grp) * DJ
                            dst = ps[32 * grp : 32 * grp + 32, :]
                            rhsh = x2ht[:, r0 : r0 + RSPLIT, j0 : j0 + NS]
                            rhsl = x2lt[:, r0 : r0 + RSPLIT, j0 : j0 + NS]
                            tp = (0, 32 * grp)
                            nc.tensor.matmul(
                                dst, x1ht[:, blk, :], rhsh,
                                start=True, stop=(passes == 1),
                                tile_position=tp, skip_group_check=True,
                            )
                            if passes == 3:
                                nc.tensor.matmul(
                                    dst, x1ht[:, blk, :], rhsl,
                                    start=False, stop=False,
                                    tile_position=tp, skip_group_check=True,
                                )
                                nc.tensor.matmul(
                                    dst, x1lt[:, blk, :], rhsh,
                                    start=False, stop=True,
                                    tile_position=tp, skip_group_check=True,
                                )
                        # Split the PSUM->SBUF copy between DVE and ACT.
                        base = qoff + h * NCOL
                        dcols = min(dve_cols, NCOL)
                        nc.vector.tensor_copy(g[:, base : base + dcols], ps[:, :dcols])
                        if dcols < NCOL:
                            nc.scalar.copy(
                                g[:, base + dcols : base + NCOL], ps[:, dcols:NCOL]
                            )
                    if quad == b0 + qb - 1:
                        off = b0 * 2 * NCOL
                        eng = nc.scalar if (alt_dge and (b0 // qb) % 2) else nc.sync
                        eng.dma_start(
                            gout[:, off : off + qb * 2 * NCOL], g[:]
                        )
    nc.compile()
    _NC_CACHE[key] = nc
    return nc


def _hilo(a):
    h = a.astype(np.float16)
    l = (a - h.astype(np.float32)).astype(np.float16)
    return h, l


def _shard_inputs(x1, x2):
    """Per-core inputs: core k -> batch k//2, row-half k%2 (halo prepadded)."""
    in_maps = []
    for k in range(N_CORES):
        b, half = k // 2, k % 2
        i0 = half * ROWS
        x1s = np.ascontiguousarray(
            x1[b, :, i0 : i0 + ROWS, :]
            .reshape(C, NBI, DI, NBJ, DJ)
            .transpose(0, 1, 3, 2, 4)
            .reshape(C, NBI * NBJ, DI * DJ)
        )
        x2s = np.zeros((C, HROWS, PW), dtype=np.float32)
        lo = max(0, PAD - i0)  # first valid padded row
        hi = min(HROWS, H + PAD - i0)  # one past last valid padded row
        x2s[:, lo:hi, PAD : PAD + W] = x2[b, :, i0 - PAD + lo : i0 - PAD + hi, :]
        x1h, x1l = _hilo(x1s)
        x2h, x2l = _hilo(x2s)
        in_maps.append({"x1h": x1h, "x1l": x1l, "x2h": x2h, "x2l": x2l})
    return in_maps


# Band-extraction index arrays (built once).  Gram partition p = 32*grp +
# il*DJ + jl; free f = (il+u)*NS + (jl+v).
_G = np.arange(4).reshape(4, 1, 1, 1, 1)
_IL = np.arange(DI).reshape(1, DI, 1, 1, 1)
_JL = np.arange(DJ).reshape(1, 1, DJ, 1, 1)
_U = np.arange(WIN).reshape(1, 1, 1, WIN, 1)
_V = np.arange(WIN).reshape(1, 1, 1, 1, WIN)


def _extract_core_output(gout_np):
    """[NQUAD, 128, 672] Gram tiles -> [441, ROWS, W] correlation output."""
    g = gout_np.reshape(NBI, NQJ, 4, DI, DJ, NR, NS)
    band = g[:, :, _G, _IL, _JL, _IL + _U, _JL + _V]  # (NBI,NQJ,4,DI,DJ,WIN,WIN)
    # -> (u, v, bi, il, qj, grp, jl) -> (441, ROWS, W)
    return band.transpose(5, 6, 0, 3, 1, 2, 4).reshape(WIN * WIN, ROWS, W)


def kernel(x1: np.ndarray, x2: np.ndarray) -> np.ndarray:
    x1 = np.asarray(x1, dtype=np.float32)
    x2 = np.asarray(x2, dtype=np.float32)
    nc = _build_nc()
    in_maps = _shard_inputs(x1, x2)
    # Retry once: a freshly-claimed device occasionally reports a transient
    # NRT_EXEC_UNIT_UNRECOVERABLE on the first execution.
    try:
        res = run_bass_kernel_spmd(nc, in_maps, core_ids=list(range(N_CORES)))
    except Exception:
        import time as _time

        _time.sleep(5.0)
        res = run_bass_kernel_spmd(nc, in_maps, core_ids=list(range(N_CORES)))
    out = np.empty((B, WIN * WIN, H, W), dtype=np.float32)
    for k in range(N_CORES):
        b, half = k // 2, k % 2
        i0 = half * ROWS
        gnp = (
            res.results[k]["gout"].reshape(128, NQUAD, 2 * NCOL).transpose(1, 0, 2)
        )
        out[b, :, i0 : i0 + ROWS, :] = _extract_core_output(gnp)
    return out



# revision 2
# speedup vs baseline: 1.5357x; 1.5357x over previous
"""Trainium2 Bass kernel for the FlowNet-style correlation module.

out[b, u*21+v, i, j] = sum_c x1[b,c,i,j] * x2pad[b,c,i+u,j+v]
with x1, x2: [4, 128, 128, 128] fp32, pad=10, window 21x21 (441 output channels).

Strategy
--------
Sharding: 8 cores = (batch 4) x (H halves). Each core handles one batch's
64-row slab: x1 slice [C=128, 64, 128] and a host-prepadded x2 slice
[C=128, 84, 148] (rows/cols include the +-10 zero halo).

Per core the correlation is computed as blocked Gram matmuls on the tensor
engine: each 8x8 pixel block of x1 (M=64) is a stationary operand on one
64-column half of the PE array (tile_position=(0,64g)); two such blocks run
concurrently against their shared 28x28 x2pad halo window (NFREE=784, split
into two 392-column PSUM passes).

Inputs are rounded to fp16 on the host and the Gram is computed in a SINGLE
fp16 pass (products accumulate in fp32 PSUM). Worst-case scale-relative
error measured at ~4.5e-4 (gate is 2e-2): the 3-pass hi/lo split of the
previous revision bought 2.9e-7 accuracy the gate does not need, at 3x the
PE time and 2x the input bytes.

The Gram tiles are converted to fp16 by the PSUM->SBUF copies (split
between DVE and ACT) and shipped to the host, which extracts each pixel's
21x21 band while unsharding. Shipped bytes per core: 12.85MB out + 4.8MB in
(vs 22MB + 10.6MB for the fp32 3-pass revision); the kernel is DMA-bound at
~360GB/s with the serial-PE matmul span (~42us) hidden underneath.
"""

import numpy as np

import concourse.mybir as mybir
import concourse.tile as tile
from concourse import bacc
from concourse.bass_utils import run_bass_kernel_spmd

# Problem constants (hardcoded; kernel.py must be self-contained).
B, C, H, W = 4, 128, 128, 128
PAD = 10
WIN = 21  # correlation window side; WIN**2 = 441 output channels
N_CORES = 8
ROWS = H // 2  # 64 output rows per core
HROWS = ROWS + 2 * PAD  # 84 x2pad rows per core
PW = W + 2 * PAD  # 148 x2pad cols

# Pixel blocking: M-block = DI x DJ = 64 pixels on one PE column half;
# 2 blocks (one pair) run concurrently on the two halves.
DI, DJ = 8, 8
NR, NS = DI + WIN - 1, DJ + WIN - 1  # 28, 28
NBI, NBJ = ROWS // DI, W // DJ  # 8, 16
NBLK = NBI * NBJ  # 128 blocks per core
NPAIR = NBLK // 2  # 64 pairs (2 blocks stacked on PSUM partitions)
NFREE = NR * NS  # 784 Gram columns per block
RSPLIT = NR // 2  # 14 rows -> 392 columns per matmul (PSUM bank: 512 fp32)
NCOL = RSPLIT * NS  # 392

F32 = mybir.dt.float32
F16 = mybir.dt.float16

_NC_CACHE = {}

# Tunables (overridable via _build_nc kwargs for experiments).
GRAM_BUFS = 6
PSUM_BUFS = 8
DVE_COLS = 184  # columns of each 392-col PSUM tile copied by DVE (rest: ACT)
# Input chunking by block-row bi: ship the x1 blocks of each group and the
# x2pad rows they need, first group small so the PE starts early.
BI_GROUPS = [(0, 1), (1, 3), (3, 5), (5, 8)]

QBATCH = 4  # pairs per output DMA (~802KB transfers)
QSCHED = [4] * 16


def _qsched(qbatch):
    if qbatch is None:
        return list(QSCHED)
    return [qbatch] * (NPAIR // qbatch)


def _build_nc(
    gram_bufs=None, psum_bufs=None, dve_cols=None, bi_groups=None, qbatch=None,
):
    gram_bufs = GRAM_BUFS if gram_bufs is None else gram_bufs
    psum_bufs = PSUM_BUFS if psum_bufs is None else psum_bufs
    dve_cols = DVE_COLS if dve_cols is None else dve_cols
    bi_groups = BI_GROUPS if bi_groups is None else bi_groups
    qsched = _qsched(qbatch)
    assert sum(qsched) == NPAIR
    key = (gram_bufs, psum_bufs, dve_cols, tuple(bi_groups), tuple(qsched))
    if key in _NC_CACHE:
        return _NC_CACHE[key]
    nc = bacc.Bacc("TRN2", target_bir_lowering=False, debug=False, num_devices=N_CORES)
    # x1 arrives host-rearranged so each 8x8 block's 64 pixels are contiguous
    # (the matmul stationary operand AP must have a single free dimension).
    x1hd = nc.dram_tensor("x1h", [C, NBLK, DI * DJ], F16, kind="ExternalInput")
    x2hd = nc.dram_tensor("x2h", [C, HROWS, PW], F16, kind="ExternalInput")
    # Flat [partition, pair-major columns] layout: pair p's Gram tile lives at
    # columns [p*NFREE, (p+1)*NFREE) regardless of the DMA batch schedule.
    gout = nc.dram_tensor("gout", [128, NPAIR * NFREE], F16, kind="ExternalOutput")

    with tile.TileContext(nc) as tc:
        with (
            tc.tile_pool(name="inp", bufs=1) as inp,
            tc.tile_pool(name="gram", bufs=gram_bufs) as gp,
            tc.tile_pool(name="psum", bufs=psum_bufs, space="PSUM") as pp,
        ):
            x1ht = inp.tile([C, NBLK, DI * DJ], F16)
            x2ht = inp.tile([C, HROWS, PW], F16)
            # Chunked input loads (x1 blocks + the x2 rows they need first).
            rprev = 0
            for glo, ghi in bi_groups:
                blo, bhi = glo * NBJ, ghi * NBJ
                rhi = min(HROWS, (ghi - 1) * DI + NR)
                nc.sync.dma_start(x1ht[:, blo:bhi, :], x1hd[:, blo:bhi, :])
                nc.sync.dma_start(x2ht[:, rprev:rhi, :], x2hd[:, rprev:rhi, :])
                rprev = rhi

            # Map pair index -> (batch start pair, batch size)
            pstart = {}
            p0 = 0
            for qb in qsched:
                for p in range(p0, p0 + qb):
                    pstart[p] = (p0, qb)
                p0 += qb
            g = None
            for bi in range(NBI):
                r0 = bi * DI
                for pj in range(NBJ // 2):
                    pair = bi * (NBJ // 2) + pj
                    b0, qb = pstart[pair]
                    if pair == b0:
                        g = gp.tile([128, qb * NFREE], F16, tag="g")
                    qoff = (pair - b0) * NFREE
                    for h in range(2):
                        ps = pp.tile([128, NCOL], F32, tag="ps")
                        rh = r0 + h * RSPLIT
                        for grp in range(2):
                            blk = bi * NBJ + 2 * pj + grp
                            j0 = (2 * pj + grp) * DJ
                            nc.tensor.matmul(
                                ps[64 * grp : 64 * grp + 64, :],
                                x1ht[:, blk, :],
                                x2ht[:, rh : rh + RSPLIT, j0 : j0 + NS],
                                start=True, stop=True,
                                tile_position=(0, 64 * grp), skip_group_check=True,
                            )
                        # Split the PSUM->SBUF (fp32->fp16) copy between DVE
                        # and ACT.
                        base = qoff + h * NCOL
                        dcols = min(dve_cols, NCOL)
                        nc.vector.tensor_copy(g[:, base : base + dcols], ps[:, :dcols])
                        if dcols < NCOL:
                            nc.scalar.copy(
                                g[:, base + dcols : base + NCOL], ps[:, dcols:NCOL]
                            )
                    if pair == b0 + qb - 1:
                        off = b0 * NFREE
                        nc.sync.dma_start(gout[:, off : off + qb * NFREE], g[:])
    nc.compile()
    _NC_CACHE[key] = nc
    return nc


def _shard_inputs(x1, x2):
    """Per-core inputs: core k -> batch k//2, row-half k%2 (halo prepadded)."""
    in_maps = []
    for k in range(N_CORES):
        b, half = k // 2, k % 2
        i0 = half * ROWS
        x1s = np.ascontiguousarray(
            x1[b, :, i0 : i0 + ROWS, :]
            .reshape(C, NBI, DI, NBJ, DJ)
            .transpose(0, 1, 3, 2, 4)
            .reshape(C, NBLK, DI * DJ)
        ).astype(np.float16)
        x2s = np.zeros((C, HROWS, PW), dtype=np.float16)
        lo = max(0, PAD - i0)  # first valid padded row
        hi = min(HROWS, H + PAD - i0)  # one past last valid padded row
        x2s[:, lo:hi, PAD : PAD + W] = x2[b, :, i0 - PAD + lo : i0 - PAD + hi, :]
        in_maps.append({"x1h": x1s, "x2h": x2s})
    return in_maps


# Band-extraction index arrays (built once).  Gram partition p = 64*grp +
# il*DJ + jl; free f = (il+u)*NS + (jl+v).
_IL = np.arange(DI).reshape(DI, 1, 1, 1)
_JL = np.arange(DJ).reshape(1, DJ, 1, 1)
_U = np.arange(WIN).reshape(1, 1, WIN, 1)
_V = np.arange(WIN).reshape(1, 1, 1, WIN)


def _extract_core_output(gout_np):
    """[128, NPAIR*NFREE] fp16 Gram tiles -> [441, ROWS, W] fp32 output."""
    # axes: (grp, il, jl, pair, r, s)
    g = gout_np.reshape(2, DI, DJ, NPAIR, NR, NS)
    # advanced-index the band; result axes (il, jl, u, v) + slices (grp, pair)
    band = g[:, _IL, _JL, :, _IL + _U, _JL + _V]  # (DI, DJ, WIN, WIN, 2, NPAIR)
    # pair = bi*(NBJ//2) + pj; block col bj = 2*pj + grp; j = bj*DJ + jl
    band = band.reshape(DI, DJ, WIN, WIN, 2, NBI, NBJ // 2)
    # -> (u, v, bi, il, pj, grp, jl) so rows = (bi,il), cols = (pj,grp,jl)
    out = band.transpose(2, 3, 5, 0, 6, 4, 1).reshape(WIN * WIN, ROWS, W)
    return out.astype(np.float32)


def kernel(x1: np.ndarray, x2: np.ndarray) -> np.ndarray:
    x1 = np.asarray(x1, dtype=np.float32)
    x2 = np.asarray(x2, dtype=np.float32)
    nc = _build_nc()
    in_maps = _shard_inputs(x1, x2)
    # Retry once: a freshly-claimed device occasionally reports a transient
    # NRT_EXEC_UNIT_UNRECOVERABLE on the first execution.
    try:
        res = run_bass_kernel_spmd(nc, in_maps, core_ids=list(range(N_CORES)))
    except Exception:
        import time as _time

        _time.sleep(5.0)
        res = run_bass_kernel_spmd(nc, in_maps, core_ids=list(range(N_CORES)))
    out = np.empty((B, WIN * WIN, H, W), dtype=np.float32)
    for k in range(N_CORES):
        b, half = k // 2, k % 2
        i0 = half * ROWS
        out[b, :, i0 : i0 + ROWS, :] = _extract_core_output(res.results[k]["gout"])
    return out


# revision 5
# speedup vs baseline: 1.5723x; 1.0239x over previous
"""Trainium2 Bass kernel for the FlowNet-style correlation module.

out[b, u*21+v, i, j] = sum_c x1[b,c,i,j] * x2pad[b,c,i+u,j+v]
with x1, x2: [4, 128, 128, 128] fp32, pad=10, window 21x21 (441 output channels).

Strategy
--------
Sharding: 8 cores = (batch 4) x (H halves). Each core handles one batch's
64-row slab: x1 slice [C=128, 64, 128] and an x2 slice [C=128, 74, 148]
(the 74 valid halo rows, cols prepadded to +-10; the 10 all-zero halo rows
are memset on device). Top-half cores get their slab flipped vertically on
the host so every core's zero band sits at the bottom and all 8 cores run
one identical SPMD program; the host un-flips while unsharding
(u -> 20-u, i -> 63-i').

Per core the correlation is computed as blocked Gram matmuls on the tensor
engine: each 8x8 pixel block of x1 (M=64) is a stationary operand on one
64-column half of the PE array (tile_position=(0,64g)); two such blocks run
concurrently against their shared 28x28 x2pad halo window (NFREE=784, split
into two 392-column PSUM banks of one 2-bank PSUM tile).

Inputs are rounded to fp16 on the host and the Gram is computed in a SINGLE
fp16 pass (products accumulate in fp32 PSUM). Worst-case scale-relative
error measured at ~4.5e-4 (gate is 2e-2): a 3-pass hi/lo split would buy
2.9e-7 accuracy the gate does not need, at 3x the PE time and 2x the input
bytes.

Each pair's two PSUM banks are copied to SBUF as fp16 by ONE DVE + ONE ACT
instruction (two-dim APs over both banks; per-instruction PSUM access
latency is ~120-170 cycles, so fewer+bigger copies pace well above the
DMA drain rate). The host extracts each pixel's 21x21 band from the fp16
Gram tiles while unsharding. Shipped bytes per core: 12.85MB out + 4.8MB
in; the kernel is DMA-bound at the cost model's 360GB/s with the serial-PE
matmul span (~42us) hidden underneath.
"""

import numpy as np

import concourse.mybir as mybir
import concourse.tile as tile
from concourse import bacc
from concourse.bass_utils import run_bass_kernel_spmd

# Problem constants (hardcoded; kernel.py must be self-contained).
B, C, H, W = 4, 128, 128, 128
PAD = 10
WIN = 21  # correlation window side; WIN**2 = 441 output channels
N_CORES = 8
ROWS = H // 2  # 64 output rows per core
HROWS = ROWS + 2 * PAD  # 84 x2pad rows per core
VROWS = HROWS - PAD  # 74 rows shipped (the other 10 are zeros, memset)
PW = W + 2 * PAD  # 148 x2pad cols

# Pixel blocking: M-block = DI x DJ = 64 pixels on one PE column half;
# 2 blocks (one pair) run concurrently on the two halves.
DI, DJ = 8, 8
NR, NS = DI + WIN - 1, DJ + WIN - 1  # 28, 28
NBI, NBJ = ROWS // DI, W // DJ  # 8, 16
NBLK = NBI * NBJ  # 128 blocks per core
NPAIR = NBLK // 2  # 64 pairs (2 blocks stacked on PSUM partitions)
NFREE = NR * NS  # 784 Gram columns per block
RSPLIT = NR // 2  # 14 rows -> 392 columns per matmul (PSUM bank: 512 fp32)
NCOL = RSPLIT * NS  # 392
BANK = 512  # fp32 elements per PSUM bank

F32 = mybir.dt.float32
F16 = mybir.dt.float16

_NC_CACHE = {}

# Tunables (overridable via _build_nc kwargs for experiments).
GRAM_BUFS = 6
PSUM_BUFS = 4  # [128, 2, 512] tiles = 2 banks each
DVE_COLS = 196  # columns of each 392-col bank copied by DVE (rest: ACT)
# Input chunks: (x1 block range, x2 shipped-row range). First chunks small
# so the PE starts early; x2 rows for block-row bi are [8bi, 8bi+28).
IN_CHUNKS = [
    ((0, 16), (0, 14)),
    (None, (14, 28)),
    ((16, 48), (28, 44)),
    ((48, 80), (44, 60)),
    ((80, 128), (60, 74)),
]

QBATCH = 4  # pairs per output DMA (~802KB transfers)
QSCHED = [4] * 16


def _qsched(qbatch):
    if qbatch is None:
        return list(QSCHED)
    return [qbatch] * (NPAIR // qbatch)


def _build_nc(
    gram_bufs=None, psum_bufs=None, dve_cols=None, in_chunks=None, qbatch=None,
    interleave=2,
):
    gram_bufs = GRAM_BUFS if gram_bufs is None else gram_bufs
    psum_bufs = PSUM_BUFS if psum_bufs is None else psum_bufs
    dve_cols = DVE_COLS if dve_cols is None else dve_cols
    in_chunks = IN_CHUNKS if in_chunks is None else in_chunks
    qsched = _qsched(qbatch)
    assert sum(qsched) == NPAIR
    key = (
        gram_bufs, psum_bufs, dve_cols,
        tuple((tuple(a) if a else None, tuple(b)) for a, b in in_chunks),
        tuple(qsched), interleave,
    )
    if key in _NC_CACHE:
        return _NC_CACHE[key]
    nc = bacc.Bacc("TRN2", target_bir_lowering=False, debug=False, num_devices=N_CORES)
    # x1 arrives host-rearranged so each 8x8 block's 64 pixels are contiguous
    # (the matmul stationary operand AP must have a single free dimension).
    x1hd = nc.dram_tensor("x1h", [C, NBLK, DI * DJ], F16, kind="ExternalInput")
    x2hd = nc.dram_tensor("x2h", [C, VROWS, PW], F16, kind="ExternalInput")
    # Flat [partition, pair-major columns] layout: pair p's Gram tile lives at
    # columns [p*NFREE, (p+1)*NFREE) regardless of the DMA batch schedule.
    gout = nc.dram_tensor("gout", [128, NPAIR * NFREE], F16, kind="ExternalOutput")

    with tile.TileContext(nc) as tc:
        with (
            tc.tile_pool(name="inp", bufs=1) as inp,
            tc.tile_pool(name="gram", bufs=gram_bufs) as gp,
            tc.tile_pool(name="psum", bufs=psum_bufs, space="PSUM") as pp,
        ):
            x1ht = inp.tile([C, NBLK, DI * DJ], F16)
            x2ht = inp.tile([C, HROWS, PW], F16)
            # The 10 halo rows every core's slab ends with are all-zero:
            # memset once (Pool is otherwise idle) instead of shipping them.
            nc.gpsimd.memset(x2ht[:, VROWS:, :], 0.0)
            for x1rng, (rlo, rhi) in in_chunks:
                if x1rng is not None:
                    blo, bhi = x1rng
                    nc.sync.dma_start(x1ht[:, blo:bhi, :], x1hd[:, blo:bhi, :])
                nc.sync.dma_start(x2ht[:, rlo:rhi, :], x2hd[:, rlo:rhi, :])

            # Map pair index -> (batch start pair, batch size)
            pstart = {}
            p0 = 0
            for qb in qsched:
                for p in range(p0, p0 + qb):
                    pstart[p] = (p0, qb)
                p0 += qb

            def emit_half(pair, h, ps):
                bi, pj = divmod(pair, NBJ // 2)
                rh = bi * DI + h * RSPLIT
                for grp in range(2):
                    blk = bi * NBJ + 2 * pj + grp
                    j0 = (2 * pj + grp) * DJ
                    nc.tensor.matmul(
                        ps[64 * grp : 64 * grp + 64, h, :NCOL],
                        x1ht[:, blk, :],
                        x2ht[:, rh : rh + RSPLIT, j0 : j0 + NS],
                        start=True, stop=True,
                        tile_position=(0, 64 * grp), skip_group_check=True,
                    )

            def emit_copies(pair, ps, g):
                b0, qb = pstart[pair]
                qi = pair - b0
                d = min(dve_cols, NCOL)
                # One DVE + one ACT copy per pair, each spanning both PSUM
                # banks with a 2-dim AP (fp32 -> fp16).
                nc.vector.tensor_copy(g[:, qi, :, :d], ps[:, :, :d])
                if d < NCOL:
                    nc.scalar.copy(g[:, qi, :, d:NCOL], ps[:, :, d:NCOL])

            tiles = {}
            for group in range(0, NPAIR, interleave):
                pairs = range(group, min(group + interleave, NPAIR))
                # h-major matmul order within the group: all h0 halves first,
                # so the PE is not head-of-line blocked on the x2 rows the h1
                # halves need while an input chunk is still in flight.
                for h in range(2):
                    for pair in pairs:
                        b0, qb = pstart[pair]
                        if h == 0:
                            if pair == b0:
                                tiles[b0] = gp.tile(
                                    [128, qb, 2, NCOL], F16, tag="g", name="g"
                                )
                            ps = pp.tile([128, 2, BANK], F32, tag="ps", name="ps")
                            tiles[("ps", pair)] = ps
                        emit_half(pair, h, tiles[("ps", pair)])
                for pair in pairs:
                    b0, qb = pstart[pair]
                    emit_copies(pair, tiles.pop(("ps", pair)), tiles[b0])
                    if pair == b0 + qb - 1:
                        off = b0 * NFREE
                        nc.sync.dma_start(
                            gout[:, off : off + qb * NFREE], tiles.pop(b0)[:]
                        )
    nc.compile()
    _NC_CACHE[key] = nc
    return nc


def _shard_inputs(x1, x2):
    """Per-core inputs: core k -> batch k//2, row-half k%2.

    Half-0 cores get their slab flipped vertically so the all-zero halo rows
    sit at the bottom for every core (identical SPMD program); the host
    un-flips during extraction.
    """
    in_maps = []
    for k in range(N_CORES):
        b, half = k // 2, k % 2
        x1s = x1[b, :, 0:ROWS, :] if half == 0 else x1[b, :, ROWS:H, :]
        if half == 0:
            x1s = x1s[:, ::-1, :]
        x1s = np.ascontiguousarray(
            x1s.reshape(C, NBI, DI, NBJ, DJ)
            .transpose(0, 1, 3, 2, 4)
            .reshape(C, NBLK, DI * DJ)
        ).astype(np.float16)
        x2s = np.zeros((C, VROWS, PW), dtype=np.float16)
        if half == 0:
            # shipped row r = image row 73 - r (slab flipped; zeros beyond)
            x2s[:, :, PAD : PAD + W] = x2[b, :, VROWS - 1 :: -1, :]
        else:
            # shipped row q = image row 54 + q
            x2s[:, :, PAD : PAD + W] = x2[b, :, H - VROWS :, :]
        in_maps.append({"x1h": x1s, "x2h": x2s})
    return in_maps


# Band-extraction index arrays (built once).  Gram partition p = 64*grp +
# il*DJ + jl; free f = (il+u)*NS + (jl+v).
_IL = np.arange(DI).reshape(DI, 1, 1, 1)
_JL = np.arange(DJ).reshape(1, DJ, 1, 1)
_U = np.arange(WIN).reshape(1, 1, WIN, 1)
_V = np.arange(WIN).reshape(1, 1, 1, WIN)


def _extract_core_output(gout_np, flipped):
    """[128, NPAIR*NFREE] fp16 Gram tiles -> [441, ROWS, W] fp32 output."""
    # axes: (grp, il, jl, pair, r, s)
    g = gout_np.reshape(2, DI, DJ, NPAIR, NR, NS)
    # advanced-index the band; result axes (il, jl, u, v) + slices (grp, pair)
    band = g[:, _IL, _JL, :, _IL + _U, _JL + _V]  # (DI, DJ, WIN, WIN, 2, NPAIR)
    # pair = bi*(NBJ//2) + pj; block col bj = 2*pj + grp; j = bj*DJ + jl
    band = band.reshape(DI, DJ, WIN, WIN, 2, NBI, NBJ // 2)
    # -> (u, v, bi, il, pj, grp, jl) so rows = (bi,il), cols = (pj,grp,jl)
    out = band.transpose(2, 3, 5, 0, 6, 4, 1).reshape(WIN, WIN, ROWS, W)
    if flipped:
        # device computed the vertically-flipped slab: u' = 20-u, i' = 63-i
        out = out[::-1, :, ::-1, :]
    return out.reshape(WIN * WIN, ROWS, W).astype(np.float32)


def kernel(x1: np.ndarray, x2: np.ndarray) -> np.ndarray:
    x1 = np.asarray(x1, dtype=np.float32)
    x2 = np.asarray(x2, dtype=np.float32)
    nc = _build_nc()
    in_maps = _shard_inputs(x1, x2)
    # Retry once: a freshly-claimed device occasionally reports a transient
    # NRT_EXEC_UNIT_UNRECOVERABLE on the first execution.
    try:
        res = run_bass_kernel_spmd(nc, in_maps, core_ids=list(range(N_CORES)))
    except Exception:
        import time as _time

        _time.sleep(5.0)
        res = run_bass_kernel_spmd(nc, in_maps, core_ids=list(range(N_CORES)))
    out = np.empty((B, WIN * WIN, H, W), dtype=np.float32)
    for k in range(N_CORES):
        b, half = k // 2, k % 2
        i0 = half * ROWS
        out[b, :, i0 : i0 + ROWS, :] = _extract_core_output(
            res.results[k]["gout"], flipped=(half == 0)
        )
    return out


# revision 7
# speedup vs baseline: 1.6729x; 1.0639x over previous
"""Trainium2 Bass kernel for the FlowNet-style correlation module.

out[b, u*21+v, i, j] = sum_c x1[b,c,i,j] * x2pad[b,c,i+u,j+v]
with x1, x2: [4, 128, 128, 128] fp32, pad=10, window 21x21 (441 output channels).

Strategy
--------
Sharding: 8 cores = (batch 4) x (H halves). Each core handles one batch's
64-row slab: x1 slice [C=128, 64, 128] and an x2 slice [C=128, 74, 148]
(the 74 valid halo rows, cols prepadded to +-10). Top-half cores get their
slab flipped vertically on the host so every core's off-image halo rows sit
BELOW row 74 and all 8 cores run one identical SPMD program; the host
un-flips while unsharding (u -> 20-u, i -> 63-i').

Per core the correlation is computed as blocked Gram matmuls on the tensor
engine: each 8x8 pixel block of x1 (M=64) is a stationary operand on one
64-column half of the PE array (tile_position=(0,64g)); two such blocks run
concurrently against their shared 28x28 x2pad halo window, split row-wise
into two PSUM banks of one 2-bank PSUM tile. Block-rows 6 and 7 only have
26 / 18 valid window rows (the rest is off-image zero, never computed or
shipped), so their PSUM splits are 2x364 / 2x252 instead of 2x392.

Inputs are rounded to fp16 on the host and the Gram is computed in a SINGLE
fp16 pass (products accumulate in fp32 PSUM). Worst-case scale-relative
error measured at ~4.5e-4 (gate is 2e-2): a 3-pass hi/lo split would buy
2.9e-7 accuracy the gate does not need, at 3x the PE time and 2x the input
bytes. A short chain of warm-up matmuls on a zero tile ramps the PE out of
its cold p-state while the first input chunks are still in flight.

Each pair's two PSUM banks are copied to SBUF as fp16 by ONE DVE + ONE ACT
instruction (two-dim APs over both banks; per-instruction PSUM access
latency is ~120-170 cycles, so fewer+bigger copies pace well above the DMA
drain rate). The host extracts each pixel's 21x21 band from the fp16 Gram
tiles while unsharding. Shipped bytes per core: 12.2MB out + 4.8MB in; the
kernel is DMA-bound at the cost model's 360GB/s with the serial-PE matmul
span (~40us) hidden underneath.
"""

import numpy as np

import concourse.mybir as mybir
import concourse.tile as tile
from concourse import bacc
from concourse.bass_utils import run_bass_kernel_spmd

# Problem constants (hardcoded; kernel.py must be self-contained).
B, C, H, W = 4, 128, 128, 128
PAD = 10
WIN = 21  # correlation window side; WIN**2 = 441 output channels
N_CORES = 8
ROWS = H // 2  # 64 output rows per core
VROWS = ROWS + PAD  # 74 valid x2pad rows per core (10 more are all-zero)
PW = W + 2 * PAD  # 148 x2pad cols

# Pixel blocking: M-block = DI x DJ = 64 pixels on one PE column half;
# 2 blocks (one pair) run concurrently on the two halves.
DI, DJ = 8, 8
NR, NS = DI + WIN - 1, DJ + WIN - 1  # 28, 28
NBI, NBJ = ROWS // DI, W // DJ  # 8, 16
NBLK = NBI * NBJ  # 128 blocks per core
NPAIR = NBLK // 2  # 64 pairs (2 blocks stacked on PSUM partitions)
PPB = NBJ // 2  # 8 pairs per block-row
BANK = 512  # fp32 elements per PSUM bank

# Per block-row bi: number of valid window rows = min(NR, VROWS - 8*bi),
# split evenly across the two PSUM banks.
VR = [min(NR, VROWS - DI * bi) for bi in range(NBI)]  # 28,...,28,26,18
RSP = [v // 2 for v in VR]  # per-bank rows: 14,...,14,13,9
NCOLS = [r * NS for r in RSP]  # per-bank cols: 392,...,392,364,252
PAIR_ELS = [2 * c for c in NCOLS]  # Gram els per pair: 784,...,728,504
BI_OFF = np.concatenate([[0], np.cumsum([PPB * e for e in PAIR_ELS])])
TOTAL_ELS = int(BI_OFF[-1])  # 47488 fp16 els per partition shipped

F32 = mybir.dt.float32
F16 = mybir.dt.float16

_NC_CACHE = {}

# Tunables (overridable via _build_nc kwargs for experiments).
GRAM_BUFS = 12
PSUM_BUFS = 4  # [128, 2, 512] tiles = 2 banks each
DVE_COLS = 196  # columns of each PSUM bank copied by DVE (rest: ACT)
N_WARM = 13  # warm-up matmuls (PE p-state ramp) while inputs stream in
# Input chunks: (x1 block range, x2 shipped-row range). First chunks small
# so the PE starts early; x2 rows for block-row bi are [8bi, 8bi+28).
IN_CHUNKS = [
    ((0, 4), None),
    (None, (0, 14)),
    ((4, 16), (14, 28)),
    ((16, 48), (28, 44)),
    ((48, 80), (44, 60)),
    ((80, 128), (60, 74)),
]

QBATCH = 4  # pairs per output DMA


def _qsched(qbatch):
    return [qbatch] * (NPAIR // qbatch)


def _build_nc(
    gram_bufs=None, psum_bufs=None, dve_cols=None, in_chunks=None, qbatch=None,
    interleave=2, n_warm=None,
):
    gram_bufs = GRAM_BUFS if gram_bufs is None else gram_bufs
    psum_bufs = PSUM_BUFS if psum_bufs is None else psum_bufs
    dve_cols = DVE_COLS if dve_cols is None else dve_cols
    in_chunks = IN_CHUNKS if in_chunks is None else in_chunks
    n_warm = N_WARM if n_warm is None else n_warm
    qsched = _qsched(QBATCH if qbatch is None else qbatch)
    assert sum(qsched) == NPAIR
    key = (
        gram_bufs, psum_bufs, dve_cols,
        tuple((tuple(a) if a else None, tuple(b) if b else None) for a, b in in_chunks),
        tuple(qsched), interleave, n_warm,
    )
    if key in _NC_CACHE:
        return _NC_CACHE[key]
    nc = bacc.Bacc("TRN2", target_bir_lowering=False, debug=False, num_devices=N_CORES)
    # x1 arrives host-rearranged so each 8x8 block's 64 pixels are contiguous
    # (the matmul stationary operand AP must have a single free dimension).
    x1hd = nc.dram_tensor("x1h", [C, NBLK, DI * DJ], F16, kind="ExternalInput")
    x2hd = nc.dram_tensor("x2h", [C, VROWS, PW], F16, kind="ExternalInput")
    # Flat [partition, bi/pair-major columns] fp16 Gram output.
    gout = nc.dram_tensor("gout", [128, TOTAL_ELS], F16, kind="ExternalOutput")

    with tile.TileContext(nc) as tc:
        with (
            tc.tile_pool(name="inp", bufs=1) as inp,
            tc.tile_pool(name="gram", bufs=gram_bufs) as gp,
            tc.tile_pool(name="psum", bufs=psum_bufs, space="PSUM") as pp,
        ):
            x1ht = inp.tile([C, NBLK, DI * DJ], F16)
            x2ht = inp.tile([C, VROWS, PW], F16)
            # Warm-up: ramp the PE p-state on an all-zero tile while the
            # first input chunks are still streaming in.
            if n_warm:
                wt = inp.tile([128, NCOLS[0]], F16)
                nc.vector.memset(wt[:, :], 0.0)
                wps = pp.tile([128, 2, BANK], F32, tag="ps", name="wps")
                for _ in range(n_warm):
                    nc.tensor.matmul(
                        wps[0:64, 0, : NCOLS[0]], wt[:, :64], wt[:, :],
                        start=True, stop=True,
                        tile_position=(0, 0), skip_group_check=True,
                    )
            for x1rng, x2rng in in_chunks:
                if x1rng is not None:
                    blo, bhi = x1rng
                    nc.sync.dma_start(x1ht[:, blo:bhi, :], x1hd[:, blo:bhi, :])
                if x2rng is not None:
                    rlo, rhi = x2rng
                    nc.sync.dma_start(x2ht[:, rlo:rhi, :], x2hd[:, rlo:rhi, :])

            # Map pair index -> (batch start pair, batch size)
            pstart = {}
            p0 = 0
            for qb in qsched:
                for p in range(p0, p0 + qb):
                    pstart[p] = (p0, qb)
                p0 += qb

            def emit_half(pair, h, ps):
                bi, pj = divmod(pair, PPB)
                rh = bi * DI + h * RSP[bi]
                for grp in range(2):
                    blk = bi * NBJ + 2 * pj + grp
                    j0 = (2 * pj + grp) * DJ
                    nc.tensor.matmul(
                        ps[64 * grp : 64 * grp + 64, h, : NCOLS[bi]],
                        x1ht[:, blk, :],
                        x2ht[:, rh : rh + RSP[bi], j0 : j0 + NS],
                        start=True, stop=True,
                        tile_position=(0, 64 * grp), skip_group_check=True,
                    )

            def emit_copies(pair, ps, g):
                bi = pair // PPB
                b0, qb = pstart[pair]
                qi = pair - b0
                ncol = NCOLS[bi]
                d = min(dve_cols, ncol)
                # One DVE + one ACT copy per pair, each spanning both PSUM
                # banks with a 2-dim AP (fp32 -> fp16).
                nc.vector.tensor_copy(g[:, qi, :, :d], ps[:, :, :d])
                if d < ncol:
                    nc.scalar.copy(g[:, qi, :, d:ncol], ps[:, :, d:ncol])

            tiles = {}
            for group in range(0, NPAIR, interleave):
                pairs = range(group, min(group + interleave, NPAIR))
                # h-major matmul order within the group: all h0 halves first,
                # so the PE is not head-of-line blocked on the x2 rows the h1
                # halves need while an input chunk is still in flight.
                for h in range(2):
                    for pair in pairs:
                        b0, qb = pstart[pair]
                        bi = pair // PPB
                        if h == 0:
                            if pair == b0:
                                tiles[b0] = gp.tile(
                                    [128, qb, 2, NCOLS[bi]], F16, tag="g", name="g"
                                )
                            ps = pp.tile([128, 2, BANK], F32, tag="ps", name="ps")
                            tiles[("ps", pair)] = ps
                        emit_half(pair, h, tiles[("ps", pair)])
                for pair in pairs:
                    b0, qb = pstart[pair]
                    bi = pair // PPB
                    emit_copies(pair, tiles.pop(("ps", pair)), tiles[b0])
                    if pair == b0 + qb - 1:
                        off = int(BI_OFF[bi]) + (b0 - bi * PPB) * PAIR_ELS[bi]
                        nc.sync.dma_start(
                            gout[:, off : off + qb * PAIR_ELS[bi]], tiles.pop(b0)[:]
                        )
    nc.compile()
    _NC_CACHE[key] = nc
    return nc


def _shard_inputs(x1, x2):
    """Per-core inputs: core k -> batch k//2, row-half k%2.

    Half-0 cores get their slab flipped vertically so the all-zero halo rows
    sit at the bottom for every core (identical SPMD program); the host
    un-flips during extraction.
    """
    in_maps = []
    for k in range(N_CORES):
        b, half = k // 2, k % 2
        x1s = x1[b, :, 0:ROWS, :] if half == 0 else x1[b, :, ROWS:H, :]
        if half == 0:
            x1s = x1s[:, ::-1, :]
        x1s = np.ascontiguousarray(
            x1s.reshape(C, NBI, DI, NBJ, DJ)
            .transpose(0, 1, 3, 2, 4)
            .reshape(C, NBLK, DI * DJ)
        ).astype(np.float16)
        x2s = np.zeros((C, VROWS, PW), dtype=np.float16)
        if half == 0:
            # shipped row r = image row 73 - r (slab flipped; zeros beyond)
            x2s[:, :, PAD : PAD + W] = x2[b, :, VROWS - 1 :: -1, :]
        else:
            # shipped row q = image row 54 + q
            x2s[:, :, PAD : PAD + W] = x2[b, :, H - VROWS :, :]
        in_maps.append({"x1h": x1s, "x2h": x2s})
    return in_maps


# Band-extraction index arrays (built once).  Gram partition p = 64*grp +
# il*DJ + jl; free f = (il+u)*NS + (jl+v).
_IL = np.arange(DI).reshape(DI, 1, 1, 1)
_JL = np.arange(DJ).reshape(1, DJ, 1, 1)
_U = np.arange(WIN).reshape(1, 1, WIN, 1)
_V = np.arange(WIN).reshape(1, 1, 1, WIN)


def _extract_core_output(gout_np, flipped):
    """[128, TOTAL_ELS] fp16 Gram tiles -> [441, ROWS, W] fp32 output."""
    # Reassemble the full (zero-padded) Gram: rows >= VR[bi] are off-image.
    g = np.zeros((2, DI, DJ, NBI, PPB, NR, NS), dtype=np.float16)
    for bi in range(NBI):
        cols = gout_np[:, BI_OFF[bi] : BI_OFF[bi + 1]]
        # (part, pair, h, rsp, s) with r = h*RSP[bi] + rsp
        g[:, :, :, bi, :, : 2 * RSP[bi], :] = cols.reshape(
            2, DI, DJ, PPB, 2 * RSP[bi], NS
        )
    g = g.reshape(2, DI, DJ, NBI * PPB, NR, NS)
    # advanced-index the band; result axes (il, jl, u, v) + slices (grp, pair)
    band = g[:, _IL, _JL, :, _IL + _U, _JL + _V]  # (DI, DJ, WIN, WIN, 2, NPAIR)
    # pair = bi*PPB + pj; block col bj = 2*pj + grp; j = bj*DJ + jl
    band = band.reshape(DI, DJ, WIN, WIN, 2, NBI, PPB)
    # -> (u, v, bi, il, pj, grp, jl) so rows = (bi,il), cols = (pj,grp,jl)
    out = band.transpose(2, 3, 5, 0, 6, 4, 1).reshape(WIN, WIN, ROWS, W)
    if flipped:
        # device computed the vertically-flipped slab: u' = 20-u, i' = 63-i
        out = out[::-1, :, ::-1, :]
    return out.reshape(WIN * WIN, ROWS, W).astype(np.float32)


def kernel(x1: np.ndarray, x2: np.ndarray) -> np.ndarray:
    x1 = np.asarray(x1, dtype=np.float32)
    x2 = np.asarray(x2, dtype=np.float32)
    nc = _build_nc()
    in_maps = _shard_inputs(x1, x2)
    # Retry once: a freshly-claimed device occasionally reports a transient
    # NRT_EXEC_UNIT_UNRECOVERABLE on the first execution.
    try:
        res = run_bass_kernel_spmd(nc, in_maps, core_ids=list(range(N_CORES)))
    except Exception:
        import time as _time

        _time.sleep(5.0)
        res = run_bass_kernel_spmd(nc, in_maps, core_ids=list(range(N_CORES)))
    out = np.empty((B, WIN * WIN, H, W), dtype=np.float32)
    for k in range(N_CORES):
        b, half = k // 2, k % 2
        i0 = half * ROWS
        out[b, :, i0 : i0 + ROWS, :] = _extract_core_output(
            res.results[k]["gout"], flipped=(half == 0)
        )
    return out


# revision 13
# speedup vs baseline: 1.7822x; 1.0653x over previous
"""Trainium2 Bass kernel for the FlowNet-style correlation module.

out[b, u*21+v, i, j] = sum_c x1[b,c,i,j] * x2pad[b,c,i+u,j+v]
with x1, x2: [4, 128, 128, 128] fp32, pad=10, window 21x21 (441 output channels).

Strategy
--------
Sharding: 8 cores = (batch 4) x (H halves). Each core handles one batch's
64-row slab: x1 slice [C=128, 64, 128] and the UNPADDED x2 slice
[C=128, 74, 128] (the 74 rows its windows can touch). Top-half cores get
their slab flipped vertically on the host so every core's off-image rows
sit past row 74 and all 8 cores run one identical SPMD program; the host
un-flips while unsharding (u -> 20-u, i -> 63-i').

Per core the correlation is computed as blocked Gram matmuls on the tensor
engine: each 8x8 pixel block of x1 (M=64) is a stationary operand on one
64-column half of the PE array (tile_position=(0,64g)). Pixel-block column
bj=pj is paired with its mirror bj=15-pj so both halves of a pair share the
same VALID window width: the zero halo (rows past the slab, columns past
the image edge) is never multiplied, never stored, never shipped — the
host zero-fills those band positions during extraction. Window rows are
split across the two banks of a 2-bank PSUM tile (up to 2x392 fp32).

Inputs are rounded to fp16 on the host and the Gram is computed in a SINGLE
fp16 pass (products accumulate in fp32 PSUM). Worst-case scale-relative
error measured at ~4.5e-4 (gate is 2e-2): a 3-pass hi/lo split would buy
2.9e-7 accuracy the gate does not need, at 3x the PE time and 2x the input
bytes. A chain of warm-up matmuls on a zero tile ramps the PE out of its
cold p-state while the first input chunks are in flight, and a 1-element
dummy ACT op hoists the lazy 1.3us ACT table load to t~0.

Each pair's two PSUM banks are copied to SBUF as fp16 by ONE DVE + ONE ACT
instruction (two-dim APs over both banks, split so both engines finish
together). The host extracts each pixel's 21x21 band from the fp16 Gram
tiles while unsharding. Shipped bytes per core: 11.5MB out + 4.4MB in; the
kernel is DMA-bound at the cost model's 360GB/s with the serial-PE matmul
span (~37us) hidden underneath.
"""

import numpy as np

import concourse.mybir as mybir
import concourse.tile as tile
from concourse import bacc, bass
from concourse.bass_utils import run_bass_kernel_spmd

# Problem constants (hardcoded; kernel.py must be self-contained).
B, C, H, W = 4, 128, 128, 128
PAD = 10
WIN = 21  # correlation window side; WIN**2 = 441 output channels
N_CORES = 8
ROWS = H // 2  # 64 output rows per core
VROWS = ROWS + PAD  # 74 x2 rows a core's windows can touch
BANK = 512  # fp32 elements per PSUM bank

# Pixel blocking: M-block = DI x DJ = 64 pixels on one PE column half;
# 2 mirrored blocks (one pair) run concurrently on the two halves.
DI, DJ = 8, 8
NR, NS = DI + WIN - 1, DJ + WIN - 1  # 28, 28
NBI, NBJ = ROWS // DI, W // DJ  # 8, 16
NBLK = NBI * NBJ  # 128 blocks per core
NPAIR = NBLK // 2  # 64 pairs (2 blocks stacked on PSUM partitions)
PPB = NBJ // 2  # 8 pairs per block-row

# Per block-row bi: valid window rows = min(NR, VROWS - 8*bi), split evenly
# across the two PSUM banks.
VRR = [min(NR, VROWS - DI * bi) for bi in range(NBI)]  # 28,...,28,26,18
RSP = [v // 2 for v in VRR]  # per-bank rows: 14,...,14,13,9
# Per pair-column pj (block bj=pj mirrored with bj=15-pj): valid window cols.
NSP = [min(NS, W + PAD - DJ * (NBJ - 1 - pj), DJ * pj + 18) for pj in range(PPB)]
# image-col start of each block's read, and the window-relative col offset
CST0 = [max(0, DJ * pj - PAD) for pj in range(PPB)]  # grp0 (bj=pj)
CST1 = [DJ * (NBJ - 1 - pj) - PAD for pj in range(PPB)]  # grp1 (bj=15-pj)
SOFF0 = [max(0, PAD - DJ * pj) for pj in range(PPB)]  # grp0 band col offset
# grp1 band col offset is always 0 (trim is on the right edge)

NCOL = [[RSP[bi] * NSP[pj] for pj in range(PPB)] for bi in range(NBI)]
PAIR_ELS = [[2 * NCOL[bi][pj] for pj in range(PPB)] for bi in range(NBI)]
_pair_off = np.concatenate([[0], np.cumsum(np.array(PAIR_ELS).reshape(-1))])
PAIR_OFF = _pair_off[:-1].reshape(NBI, PPB)  # gout col offset per pair
TOTAL_ELS = int(_pair_off[-1])

F32 = mybir.dt.float32
F16 = mybir.dt.float16

_NC_CACHE = {}

# Tunables (overridable via _build_nc kwargs for experiments).
GRAM_BUFS = 12
PSUM_BUFS = 4  # [128, 2, 512] tiles = 2 banks each


def _dve_cols(ncol):
    """DVE/ACT split of a 2-bank copy so both engines finish together."""
    return min(ncol, round(0.4444 * ncol + 20.5))


N_WARM = 16  # warm-up matmuls (PE p-state ramp) while inputs stream in
# Input chunks: (x1 block range, x2 row range). x2 rows for block-row bi
# are [8bi, 8bi+28); each chunk feeds one block-row ahead.
IN_CHUNKS = [
    ((0, 16), (0, 28)),
    (None, (28, 36)),
    ((16, 32), (36, 44)),
    ((32, 48), (44, 52)),
    ((48, 64), (52, 60)),
    ((64, 96), (60, 68)),
    ((96, 128), (68, 74)),
]

QBATCH = 4  # pairs per output DMA
QSCHED = [4] * 15 + [2, 2]


def _qsched(qbatch):
    if qbatch is None:
        return list(QSCHED)
    return [qbatch] * (NPAIR // qbatch)


def _build_nc(
    gram_bufs=None, psum_bufs=None, in_chunks=None, qbatch=None,
    interleave=2, n_warm=None,
):
    gram_bufs = GRAM_BUFS if gram_bufs is None else gram_bufs
    psum_bufs = PSUM_BUFS if psum_bufs is None else psum_bufs
    in_chunks = IN_CHUNKS if in_chunks is None else in_chunks
    n_warm = N_WARM if n_warm is None else n_warm
    qsched = _qsched(qbatch)
    assert sum(qsched) == NPAIR
    key = (
        gram_bufs, psum_bufs,
        tuple((tuple(a) if a else None, tuple(b) if b else None) for a, b in in_chunks),
        tuple(qsched), interleave, n_warm,
    )
    if key in _NC_CACHE:
        return _NC_CACHE[key]
    nc = bacc.Bacc("TRN2", target_bir_lowering=False, debug=False, num_devices=N_CORES)
    # x1 arrives host-rearranged so each 8x8 block's 64 pixels are contiguous
    # (the matmul stationary operand AP must have a single free dimension).
    x1hd = nc.dram_tensor("x1h", [C, NBLK, DI * DJ], F16, kind="ExternalInput")
    x2hd = nc.dram_tensor("x2h", [C, VROWS, W], F16, kind="ExternalInput")
    # Flat [partition, pair-major columns] fp16 Gram output.
    gout = nc.dram_tensor("gout", [128, TOTAL_ELS], F16, kind="ExternalOutput")

    with tile.TileContext(nc) as tc:
        with (
            tc.tile_pool(name="inp", bufs=1) as inp,
            tc.tile_pool(name="gram", bufs=gram_bufs) as gp,
            tc.tile_pool(name="psum", bufs=psum_bufs, space="PSUM") as pp,
        ):
            x1ht = inp.tile([C, NBLK, DI * DJ], F16)
            x2ht = inp.tile([C, VROWS, W], F16)
            # Warm-up: ramp the PE p-state on an all-zero tile while the
            # first input chunks are still streaming in, and hoist the lazy
            # ACT table load (~1.3us) to t~0 with a 1-element dummy op.
            wt = inp.tile([128, NCOL[0][-1]], F16)
            nc.vector.memset(wt[:, :], 0.0)
            nc.scalar.copy(wt[:, 0:1], wt[:, 1:2])
            if n_warm:
                wps = pp.tile([128, 2, BANK], F32, tag="ps", name="wps")
                for _ in range(n_warm):
                    nc.tensor.matmul(
                        wps[0:64, 0, : NCOL[0][-1]], wt[:, :64], wt[:, :],
                        start=True, stop=True,
                        tile_position=(0, 0), skip_group_check=True,
                    )
            for x1rng, x2rng in in_chunks:
                if x1rng is not None:
                    blo, bhi = x1rng
                    nc.sync.dma_start(x1ht[:, blo:bhi, :], x1hd[:, blo:bhi, :])
                if x2rng is not None:
                    rlo, rhi = x2rng
                    nc.sync.dma_start(x2ht[:, rlo:rhi, :], x2hd[:, rlo:rhi, :])

            # Map pair index -> (batch start pair, batch size)
            pstart = {}
            p0 = 0
            for qb in qsched:
                for p in range(p0, p0 + qb):
                    pstart[p] = (p0, qb)
                p0 += qb

            def emit_half(pair, h, ps):
                bi, pj = divmod(pair, PPB)
                rh = bi * DI + h * RSP[bi]
                for grp in range(2):
                    bj = pj if grp == 0 else NBJ - 1 - pj
                    c0 = CST0[pj] if grp == 0 else CST1[pj]
                    nc.tensor.matmul(
                        ps[64 * grp : 64 * grp + 64, h, : NCOL[bi][pj]],
                        x1ht[:, bi * NBJ + bj, :],
                        x2ht[:, rh : rh + RSP[bi], c0 : c0 + NSP[pj]],
                        start=True, stop=True,
                        tile_position=(0, 64 * grp), skip_group_check=True,
                    )

            tiles = {}
            for group in range(0, NPAIR, interleave):
                pairs = range(group, min(group + interleave, NPAIR))
                # h-major matmul order within the group: all h0 halves first,
                # so the PE is not head-of-line blocked on the x2 rows the h1
                # halves need while an input chunk is still in flight.
                for pair in pairs:
                    b0, qb = pstart[pair]
                    bi = pair // PPB
                    if pair == b0:
                        bels = sum(PAIR_ELS[bi][b0 - bi * PPB + i] for i in range(qb))
                        tiles[b0] = gp.tile([128, bels], F16, tag="g", name="g")
                    ps = pp.tile([128, 2, BANK], F32, tag="ps", name="ps")
                    tiles[("ps", pair)] = ps
                    emit_half(pair, 0, ps)
                for pair in pairs:
                    b0, qb = pstart[pair]
                    bi, pj = divmod(pair, PPB)
                    emit_half(pair, 1, tiles[("ps", pair)])
                    # copies fire right after this pair's h1 so the PSUM slot
                    # returns as early as possible
                    goff = int(PAIR_OFF[bi][pj] - PAIR_OFF[bi][b0 - bi * PPB])
                    ps = tiles.pop(("ps", pair))
                    g = tiles[b0]
                    ncol = NCOL[bi][pj]
                    d = _dve_cols(ncol)
                    # One DVE + one ACT copy per pair spanning both PSUM
                    # banks (fp32 -> fp16); the gram tile is flat because
                    # pairs in a batch have differing widths, so build the
                    # 2-dim destination APs by hand.
                    gt = g[:]
                    bels = gt.ap[0][0]
                    nc.vector.tensor_copy(
                        bass.AP(
                            tensor=gt.tensor, offset=gt.offset + goff,
                            ap=[[bels, 128], [ncol, 2], [1, d]],
                        ),
                        ps[:, :, :d],
                    )
                    if d < ncol:
                        nc.scalar.copy(
                            bass.AP(
                                tensor=gt.tensor, offset=gt.offset + goff + d,
                                ap=[[bels, 128], [ncol, 2], [1, ncol - d]],
                            ),
                            ps[:, :, d:ncol],
                        )
                    if pair == b0 + qb - 1:
                        off = int(PAIR_OFF[bi][b0 - bi * PPB])
                        nc.sync.dma_start(
                            gout[:, off : off + bels], tiles.pop(b0)[:]
                        )
    nc.compile()
    _NC_CACHE[key] = nc
    return nc


def _shard_inputs(x1, x2):
    """Per-core inputs: core k -> batch k//2, row-half k%2.

    Half-0 cores get their slab flipped vertically so the off-image halo
    rows sit past the end for every core (identical SPMD program); the host
    un-flips during extraction.
    """
    in_maps = []
    for k in range(N_CORES):
        b, half = k // 2, k % 2
        x1s = x1[b, :, 0:ROWS, :] if half == 0 else x1[b, :, ROWS:H, :]
        if half == 0:
            x1s = x1s[:, ::-1, :]
        x1s = np.ascontiguousarray(
            x1s.reshape(C, NBI, DI, NBJ, DJ)
            .transpose(0, 1, 3, 2, 4)
            .reshape(C, NBLK, DI * DJ)
        ).astype(np.float16)
        if half == 0:
            # shipped row r = image row 73 - r (slab flipped)
            x2s = np.ascontiguousarray(x2[b, :, VROWS - 1 :: -1, :]).astype(np.float16)
        else:
            # shipped row q = image row 54 + q
            x2s = np.ascontiguousarray(x2[b, :, H - VROWS :, :]).astype(np.float16)
        in_maps.append({"x1h": x1s, "x2h": x2s})
    return in_maps


# Band-extraction index arrays (built once).  Gram partition p = 64*grp +
# il*DJ + jl; full-band free f = (il+u)*NS + (jl+v).
_IL = np.arange(DI).reshape(DI, 1, 1, 1)
_JL = np.arange(DJ).reshape(1, DJ, 1, 1)
_U = np.arange(WIN).reshape(1, 1, WIN, 1)
_V = np.arange(WIN).reshape(1, 1, 1, WIN)
# output image-col for band axes (pj, grp, jl)
_JMAP = np.empty((PPB, 2, DJ), dtype=np.int64)
for _pj in range(PPB):
    _JMAP[_pj, 0] = _pj * DJ + np.arange(DJ)
    _JMAP[_pj, 1] = (NBJ - 1 - _pj) * DJ + np.arange(DJ)


def _extract_core_output(gout_np, flipped):
    """[128, TOTAL_ELS] fp16 Gram tiles -> [441, ROWS, W] fp32 output."""
    # Reassemble the full zero-padded Gram (axes grp, il, jl, bi, pj, r, s):
    # rows >= VRR[bi] and cols outside [soff, soff+NSP[pj]) are off-image.
    g = np.zeros((2, DI, DJ, NBI, PPB, NR, NS), dtype=np.float16)
    for bi in range(NBI):
        for pj in range(PPB):
            off = int(PAIR_OFF[bi][pj])
            r2, nsp = 2 * RSP[bi], NSP[pj]
            cols = gout_np[:, off : off + r2 * nsp].reshape(2, DI, DJ, r2, nsp)
            g[0, :, :, bi, pj, :r2, SOFF0[pj] : SOFF0[pj] + nsp] = cols[0]
            g[1, :, :, bi, pj, :r2, 0:nsp] = cols[1]
    # advanced-index the band; result axes (il, jl, u, v) + slices (grp, bi, pj)
    band = g[:, _IL, _JL, :, :, _IL + _U, _JL + _V]  # (DI,DJ,WIN,WIN,2,NBI,PPB)
    # -> (u, v, bi, il, pj, grp, jl); j = _JMAP[pj, grp, jl]
    band = band.transpose(2, 3, 5, 0, 6, 4, 1).reshape(WIN, WIN, ROWS, PPB * 2 * DJ)
    out = np.empty((WIN, WIN, ROWS, W), dtype=np.float16)
    out[:, :, :, _JMAP.reshape(-1)] = band
    if flipped:
        # device computed the vertically-flipped slab: u' = 20-u, i' = 63-i
        out = out[::-1, :, ::-1, :]
    return out.reshape(WIN * WIN, ROWS, W).astype(np.float32)


def kernel(x1: np.ndarray, x2: np.ndarray) -> np.ndarray:
    x1 = np.asarray(x1, dtype=np.float32)
    x2 = np.asarray(x2, dtype=np.float32)
    nc = _build_nc()
    in_maps = _shard_inputs(x1, x2)
    # Retry once: a freshly-claimed device occasionally reports a transient
    # NRT_EXEC_UNIT_UNRECOVERABLE on the first execution.
    try:
        res = run_bass_kernel_spmd(nc, in_maps, core_ids=list(range(N_CORES)))
    except Exception:
        import time as _time

        _time.sleep(5.0)
        res = run_bass_kernel_spmd(nc, in_maps, core_ids=list(range(N_CORES)))
    out = np.empty((B, WIN * WIN, H, W), dtype=np.float32)
    for k in range(N_CORES):
        b, half = k // 2, k % 2
        i0 = half * ROWS
        out[b, :, i0 : i0 + ROWS, :] = _extract_core_output(
            res.results[k]["gout"], flipped=(half == 0)
        )
    return out


# revision 16
# speedup vs baseline: 1.8118x; 1.0166x over previous
"""Trainium2 Bass kernel for the FlowNet-style correlation module.

out[b, u*21+v, i, j] = sum_c x1[b,c,i,j] * x2pad[b,c,i+u,j+v]
with x1, x2: [4, 128, 128, 128] fp32, pad=10, window 21x21 (441 output channels).

Strategy
--------
Sharding: 8 cores = (batch 4) x (H halves). Each core handles one batch's
64-row slab: x1 slice [C=128, 64, 128] and the UNPADDED x2 slice
[C=128, 74, 128] (the 74 rows its windows can touch). Top-half cores get
their slab flipped vertically on the host so every core's off-image rows
sit past row 74 and all 8 cores run one identical SPMD program; the host
un-flips while unsharding (u -> 20-u, i -> 63-i').

Per core the correlation is computed as blocked Gram matmuls on the tensor
engine over a mix of pixel-block shapes chosen to balance the serial-PE
matmul rate against the 360GB/s DMA drain rate:
  - 8x8 blocks (M=64) in MIRRORED pairs on the two PE column halves
    (tile_position=(0,64g)): column bj is paired with 15-bj so both halves
    share the same valid window width at the image edges;
  - KMIX 8x16 blocks (M=128, full PE width) per block-row in the interior:
    36% fewer PE columns per pixel for 29% more shipped Gram bytes.
The zero halo (rows past the slab, columns past the image edge) is never
multiplied, never stored, never shipped - the host zero-fills those band
positions during extraction. Window rows are split across the two banks of
a 2-bank PSUM tile (up to 2x504 fp32).

Inputs are rounded to fp16 on the host and the Gram is computed in a SINGLE
fp16 pass (products accumulate in fp32 PSUM). Worst-case scale-relative
error measured at ~4.5e-4 (gate is 2e-2): a 3-pass hi/lo split would buy
2.9e-7 accuracy the gate does not need, at 3x the PE time and 2x the input
bytes. A chain of warm-up matmuls on a zero tile ramps the PE out of its
cold p-state while the first input chunks are in flight, and a 1-element
dummy ACT op hoists the lazy 1.3us ACT table load to t~0.

Each unit's two PSUM banks are copied to SBUF as fp16 by ONE DVE + ONE ACT
instruction (two-dim APs over both banks, split so both engines finish
together). The host extracts each pixel's 21x21 band from the fp16 Gram
tiles while unsharding. Shipped bytes per core: ~12.4MB out + 4.4MB in;
the kernel is DMA-bound at the cost model's 360GB/s with the serial-PE
matmul span (~34us) hidden underneath.
"""

import numpy as np

import concourse.mybir as mybir
import concourse.tile as tile
from concourse import bacc, bass
from concourse.bass_utils import run_bass_kernel_spmd

# Problem constants (hardcoded; kernel.py must be self-contained).
B, C, H, W = 4, 128, 128, 128
PAD = 10
WIN = 21  # correlation window side; WIN**2 = 441 output channels
N_CORES = 8
ROWS = H // 2  # 64 output rows per core
VROWS = ROWS + PAD  # 74 x2 rows a core's windows can touch
BANK = 512  # fp32 elements per PSUM bank

DI = 8  # block rows
NBI, NBJ = ROWS // DI, W // 8  # 8 block-rows, 16 8-wide block-cols
NR = DI + WIN - 1  # 28 window rows

# Per block-row bi: valid window rows = min(NR, VROWS - 8*bi), split evenly
# across the two PSUM banks.
VRR = [min(NR, VROWS - DI * bi) for bi in range(NBI)]  # 28,...,28,26,18
RSP = [v // 2 for v in VRR]  # per-bank rows: 14,...,14,13,9

KMIX = 2  # 8x16 blocks per block-row (PE/DMA balance knob)

# ---------------------------------------------------------------------------
# Unit table: one entry per PSUM-tile unit, in emission order.
#   pair unit: two mirrored 8x8 blocks (M=64 each) on the PE column halves
#   wide unit: one 8x16 block (M=128) on the full PE width
# Fields: kind, bi, per-grp pixel-col starts, x2 col read range, valid
# window width nsp, band col offsets, x1/gout offsets.
# ---------------------------------------------------------------------------


def _build_units(kmix):
    kml = [kmix] * NBI if isinstance(kmix, int) else list(kmix)
    units = []
    x1_off = 0
    g_off = 0
    for bi in range(NBI):
        km = kml[bi]
        r = RSP[bi]
        seq = []
        # mirrored edge pairs (bj, 15-bj) for bj = 0, 1
        for bj in range(2):
            seq.append(("pair", bj, NBJ - 1 - bj))
        # km wide blocks over bj 2,3 / 4,5 / ...
        for wset in range(km):
            seq.append(("wide", 2 + 2 * wset, None))
        # remaining interior as mirrored pairs
        lo, hi = 2 + 2 * km, NBJ - 3
        while lo < hi:
            seq.append(("pair", lo, hi))
            lo, hi = lo + 1, hi - 1
        for kind, a, b in seq:
            if kind == "pair":
                nsp = min(28, W + PAD - 8 * b, 8 * a + 18)
                u = dict(
                    kind=kind, bi=bi, r=r, nsp=nsp, ncol=r * nsp,
                    bjs=(a, b),
                    cst=(max(0, 8 * a - PAD), 8 * b - PAD),
                    soff=(max(0, PAD - 8 * a), 0),
                    x1_off=x1_off, g_off=g_off,
                )
                x1_off += 128
                g_off += 2 * u["ncol"]
            else:
                nsp = 36
                u = dict(
                    kind=kind, bi=bi, r=r, nsp=nsp, ncol=r * nsp,
                    c0=8 * a,  # pixel-col start (block covers 16 cols)
                    cst=8 * a - PAD,
                    x1_off=x1_off, g_off=g_off,
                )
                x1_off += 128
                g_off += 2 * u["ncol"]
            units.append(u)
    return units, x1_off, g_off


UNITS, X1_ELS, TOTAL_ELS = _build_units(KMIX)
NUNIT = len(UNITS)


def set_kmix(k):
    """Experimentation hook: rebuild the unit table for a different mix."""
    global KMIX, UNITS, X1_ELS, TOTAL_ELS, NUNIT
    KMIX = tuple(k) if not isinstance(k, int) else k
    UNITS, X1_ELS, TOTAL_ELS = _build_units(k)
    NUNIT = len(UNITS)
    _NC_CACHE.clear()

F32 = mybir.dt.float32
F16 = mybir.dt.float16

_NC_CACHE = {}

# Tunables (overridable via _build_nc kwargs for experiments).
GRAM_BUFS = 12
PSUM_BUFS = 4  # [128, 2, 512] tiles = 2 banks each


def _dve_cols(ncol):
    """DVE/ACT split of a 2-bank copy so both engines finish together."""
    return min(ncol, round(0.4444 * ncol + 20.5))


N_WARM = 16  # warm-up matmuls (PE p-state ramp) while inputs stream in
# Input chunks: (x1 el range, x2 row range). x2 rows for block-row bi are
# [8bi, 8bi+28); each chunk feeds one block-row ahead.
IN_CHUNKS = [
    ((0, 1024), (0, 28)),
    (None, (28, 36)),
    ((1024, 2048), (36, 44)),
    ((2048, 3072), (44, 52)),
    ((3072, 4096), (52, 60)),
    ((4096, 6144), (60, 68)),
    ((6144, 8192), (68, 74)),
]

# DMA batches: units per output DMA (UPB units per bi; 2 DMAs per bi).
QSCHED = None  # computed: split each bi's units into two halves


def _qsched():
    out = []
    for bi in range(NBI):
        n = sum(1 for u in UNITS if u["bi"] == bi)
        out.append(n - n // 2)
        out.append(n // 2)
    return out


def _build_nc(
    gram_bufs=None, psum_bufs=None, in_chunks=None,
    interleave=2, n_warm=None,
):
    gram_bufs = GRAM_BUFS if gram_bufs is None else gram_bufs
    psum_bufs = PSUM_BUFS if psum_bufs is None else psum_bufs
    in_chunks = IN_CHUNKS if in_chunks is None else in_chunks
    n_warm = N_WARM if n_warm is None else n_warm
    qsched = _qsched()
    assert sum(qsched) == NUNIT
    key = (
        KMIX, gram_bufs, psum_bufs,
        tuple((tuple(a) if a else None, tuple(b) if b else None) for a, b in in_chunks),
        interleave, n_warm,
    )
    if key in _NC_CACHE:
        return _NC_CACHE[key]
    nc = bacc.Bacc("TRN2", target_bir_lowering=False, debug=False, num_devices=N_CORES)
    # x1 arrives host-rearranged so each block's pixels are contiguous
    # (the matmul stationary operand AP must have a single free dimension).
    x1hd = nc.dram_tensor("x1h", [C, X1_ELS], F16, kind="ExternalInput")
    x2hd = nc.dram_tensor("x2h", [C, VROWS, W], F16, kind="ExternalInput")
    # Flat [partition, unit-major columns] fp16 Gram output.
    gout = nc.dram_tensor("gout", [128, TOTAL_ELS], F16, kind="ExternalOutput")

    # unit index -> (batch start unit, batch size)
    ustart = {}
    u0 = 0
    for qb in qsched:
        for p in range(u0, u0 + qb):
            ustart[p] = (u0, qb)
        u0 += qb

    with tile.TileContext(nc) as tc:
        with (
            tc.tile_pool(name="inp", bufs=1) as inp,
            tc.tile_pool(name="gram", bufs=gram_bufs) as gp,
            tc.tile_pool(name="psum", bufs=psum_bufs, space="PSUM") as pp,
        ):
            x1ht = inp.tile([C, X1_ELS], F16)
            x2ht = inp.tile([C, VROWS, W], F16)
            # Warm-up: ramp the PE p-state on an all-zero tile while the
            # first input chunks are still streaming in, and hoist the lazy
            # ACT table load (~1.3us) to t~0 with a 1-element dummy op.
            wt = inp.tile([128, 504], F16)
            nc.vector.memset(wt[:, :], 0.0)
            nc.scalar.copy(wt[:, 0:1], wt[:, 1:2])
            if n_warm:
                wps = pp.tile([128, 2, BANK], F32, tag="ps", name="wps")
                for _ in range(n_warm):
                    nc.tensor.matmul(
                        wps[0:64, 0, :504], wt[:, :64], wt[:, :],
                        start=True, stop=True,
                        tile_position=(0, 0), skip_group_check=True,
                    )
            for x1rng, x2rng in in_chunks:
                if x1rng is not None:
                    elo, ehi = x1rng
                    nc.sync.dma_start(x1ht[:, elo:ehi], x1hd[:, elo:ehi])
                if x2rng is not None:
                    rlo, rhi = x2rng
                    nc.sync.dma_start(x2ht[:, rlo:rhi, :], x2hd[:, rlo:rhi, :])

            def emit_half(u, h, ps):
                r = u["r"]
                rh = u["bi"] * DI + h * r
                ncol = u["ncol"]
                if u["kind"] == "pair":
                    for grp in range(2):
                        c0 = u["cst"][grp]
                        nc.tensor.matmul(
                            ps[64 * grp : 64 * grp + 64, h, :ncol],
                            x1ht[:, u["x1_off"] + 64 * grp : u["x1_off"] + 64 * grp + 64],
                            x2ht[:, rh : rh + r, c0 : c0 + u["nsp"]],
                            start=True, stop=True,
                            tile_position=(0, 64 * grp), skip_group_check=True,
                        )
                else:
                    nc.tensor.matmul(
                        ps[:, h, :ncol],
                        x1ht[:, u["x1_off"] : u["x1_off"] + 128],
                        x2ht[:, rh : rh + r, u["cst"] : u["cst"] + u["nsp"]],
                        start=True, stop=True,
                        tile_position=(0, 0), skip_group_check=True,
                    )

            tiles = {}
            for group in range(0, NUNIT, interleave):
                us = range(group, min(group + interleave, NUNIT))
                # h-major matmul order within the group: all h0 halves first,
                # so the PE is not head-of-line blocked on the x2 rows the h1
                # halves need while an input chunk is still in flight.
                for ui in us:
                    b0, qb = ustart[ui]
                    if ui == b0:
                        bels = sum(2 * UNITS[b0 + i]["ncol"] for i in range(qb))
                        tiles[b0] = gp.tile([128, bels], F16, tag="g", name="g")
                        tiles[("bels", b0)] = bels
                    ps = pp.tile([128, 2, BANK], F32, tag="ps", name="ps")
                    tiles[("ps", ui)] = ps
                    emit_half(UNITS[ui], 0, ps)
                for ui in us:
                    b0, qb = ustart[ui]
                    u = UNITS[ui]
                    emit_half(u, 1, tiles[("ps", ui)])
                    # copies fire right after this unit's h1 so the PSUM slot
                    # returns as early as possible
                    ps = tiles.pop(("ps", ui))
                    g = tiles[b0]
                    ncol = u["ncol"]
                    d = _dve_cols(ncol)
                    goff = u["g_off"] - UNITS[b0]["g_off"]
                    gt = g[:]
                    bels = tiles[("bels", b0)]
                    nc.vector.tensor_copy(
                        bass.AP(
                            tensor=gt.tensor, offset=gt.offset + goff,
                            ap=[[bels, 128], [ncol, 2], [1, d]],
                        ),
                        ps[:, :, :d],
                    )
                    if d < ncol:
                        nc.scalar.copy(
                            bass.AP(
                                tensor=gt.tensor, offset=gt.offset + goff + d,
                                ap=[[bels, 128], [ncol, 2], [1, ncol - d]],
                            ),
                            ps[:, :, d:ncol],
                        )
                    if ui == b0 + qb - 1:
                        off = UNITS[b0]["g_off"]
                        nc.sync.dma_start(
                            gout[:, off : off + bels], tiles.pop(b0)[:]
                        )
                        tiles.pop(("bels", b0))
    nc.compile()
    _NC_CACHE[key] = nc
    return nc


def _shard_inputs(x1, x2):
    """Per-core inputs: core k -> batch k//2, row-half k%2.

    Half-0 cores get their slab flipped vertically so the off-image halo
    rows sit past the end for every core (identical SPMD program); the host
    un-flips during extraction.
    """
    in_maps = []
    for k in range(N_CORES):
        b, half = k // 2, k % 2
        x1s = x1[b, :, 0:ROWS, :] if half == 0 else x1[b, :, ROWS:H, :]
        if half == 0:
            x1s = x1s[:, ::-1, :]
        x1s = x1s.astype(np.float16)
        x1r = np.empty((C, X1_ELS), dtype=np.float16)
        for u in UNITS:
            i0 = u["bi"] * DI
            if u["kind"] == "pair":
                for grp, bj in enumerate(u["bjs"]):
                    blkpx = x1s[:, i0 : i0 + DI, 8 * bj : 8 * bj + 8]
                    x1r[:, u["x1_off"] + 64 * grp : u["x1_off"] + 64 * grp + 64] = (
                        blkpx.reshape(C, 64)
                    )
            else:
                blkpx = x1s[:, i0 : i0 + DI, u["c0"] : u["c0"] + 16]
                x1r[:, u["x1_off"] : u["x1_off"] + 128] = blkpx.reshape(C, 128)
        if half == 0:
            # shipped row r = image row 73 - r (slab flipped)
            x2s = np.ascontiguousarray(x2[b, :, VROWS - 1 :: -1, :]).astype(np.float16)
        else:
            # shipped row q = image row 54 + q
            x2s = np.ascontiguousarray(x2[b, :, H - VROWS :, :]).astype(np.float16)
        in_maps.append({"x1h": x1r, "x2h": x2s})
    return in_maps


# Band-extraction index arrays (built once).
_IL = np.arange(DI).reshape(DI, 1, 1, 1)
_JL8 = np.arange(8).reshape(1, 8, 1, 1)
_JL16 = np.arange(16).reshape(1, 16, 1, 1)
_U = np.arange(WIN).reshape(1, 1, WIN, 1)
_V = np.arange(WIN).reshape(1, 1, 1, WIN)


def _extract_core_output(gout_np, flipped):
    """[128, TOTAL_ELS] fp16 Gram tiles -> [441, ROWS, W] fp32 output."""
    out = np.zeros((WIN, WIN, ROWS, W), dtype=np.float16)
    for u in UNITS:
        bi, r2, nsp, ncol = u["bi"], 2 * u["r"], u["nsp"], u["ncol"]
        cols = gout_np[:, u["g_off"] : u["g_off"] + 2 * ncol]
        i0 = bi * DI
        if u["kind"] == "pair":
            # partition p = 64*grp + il*8 + jl; free f = rr*nsp + ss
            g = cols.reshape(2, DI, 8, r2, nsp)
            for grp, bj in enumerate(u["bjs"]):
                gf = np.zeros((DI, 8, NR, 28), dtype=np.float16)
                gf[:, :, :r2, u["soff"][grp] : u["soff"][grp] + nsp] = g[grp]
                band = gf[_IL, _JL8, _IL + _U, _JL8 + _V]  # (DI, 8, WIN, WIN)
                out[:, :, i0 : i0 + DI, 8 * bj : 8 * bj + 8] = band.transpose(
                    2, 3, 0, 1
                )
        else:
            # partition p = il*16 + jl16; free f = rr*36 + ss
            g = cols.reshape(DI, 16, r2, nsp)
            gf = np.zeros((DI, 16, NR, 36), dtype=np.float16)
            gf[:, :, :r2, :] = g
            band = gf[_IL, _JL16, _IL + _U, _JL16 + _V]  # (DI, 16, WIN, WIN)
            out[:, :, i0 : i0 + DI, u["c0"] : u["c0"] + 16] = band.transpose(
                2, 3, 0, 1
            )
    if flipped:
        # device computed the vertically-flipped slab: u' = 20-u, i' = 63-i
        out = out[::-1, :, ::-1, :]
    return out.reshape(WIN * WIN, ROWS, W).astype(np.float32)


def kernel(x1: np.ndarray, x2: np.ndarray) -> np.ndarray:
    x1 = np.asarray(x1, dtype=np.float32)
    x2 = np.asarray(x2, dtype=np.float32)
    nc = _build_nc()
    in_maps = _shard_inputs(x1, x2)
    # Retry once: a freshly-claimed device occasionally reports a transient
    # NRT_EXEC_UNIT_UNRECOVERABLE on the first execution.
    try:
        res = run_bass_kernel_spmd(nc, in_maps, core_ids=list(range(N_CORES)))
    except Exception:
        import time as _time

        _time.sleep(5.0)
        res = run_bass_kernel_spmd(nc, in_maps, core_ids=list(range(N_CORES)))
    out = np.empty((B, WIN * WIN, H, W), dtype=np.float32)
    for k in range(N_CORES):
        b, half = k // 2, k % 2
        i0 = half * ROWS
        out[b, :, i0 : i0 + ROWS, :] = _extract_core_output(
            res.results[k]["gout"], flipped=(half == 0)
        )
    return out


# revision 17
# speedup vs baseline: 1.8598x; 1.0265x over previous
"""Trainium2 Bass kernel for the FlowNet-style correlation module.

out[b, u*21+v, i, j] = sum_c x1[b,c,i,j] * x2pad[b,c,i+u,j+v]
with x1, x2: [4, 128, 128, 128] fp32, pad=10, window 21x21 (441 output channels).

Strategy
--------
Sharding: 8 cores = (batch 4) x (H halves). Each core handles one batch's
64-row slab: x1 slice [C=128, 64, 128] and the UNPADDED x2 slice
[C=128, 74, 128] (the 74 rows its windows can touch). Top-half cores get
their slab flipped vertically on the host so every core's off-image rows
sit past row 74 and all 8 cores run one identical SPMD program; the host
un-flips while unsharding (u -> 20-u, i -> 63-i').

Per core the correlation is computed as blocked Gram matmuls on the tensor
engine over a mix of pixel-block shapes chosen to balance the serial-PE
matmul rate against the 360GB/s DMA drain rate:
  - 8x8 blocks (M=64) in MIRRORED pairs on the two PE column halves
    (tile_position=(0,64g)): column bj is paired with 15-bj so both halves
    share the same valid window width at the image edges;
  - KMIX 8x16 blocks (M=128, full PE width) per block-row in the interior:
    36% fewer PE columns per pixel for 29% more shipped Gram bytes.
The zero halo (rows past the slab, columns past the image edge) is never
multiplied, never stored, never shipped - the host zero-fills those band
positions during extraction. Window rows are split across the two banks of
a 2-bank PSUM tile (up to 2x504 fp32).

Inputs are rounded to fp16 on the host and the Gram is computed in a SINGLE
fp16 pass (products accumulate in fp32 PSUM). Worst-case scale-relative
error measured at ~4.5e-4 (gate is 2e-2): a 3-pass hi/lo split would buy
2.9e-7 accuracy the gate does not need, at 3x the PE time and 2x the input
bytes. A chain of warm-up matmuls on a zero tile ramps the PE out of its
cold p-state while the first input chunks are in flight, and a 1-element
dummy ACT op hoists the lazy 1.3us ACT table load to t~0.

Each unit's two PSUM banks are copied to SBUF as fp16 by ONE DVE + ONE ACT
instruction (two-dim APs over both banks, split so both engines finish
together). The host extracts each pixel's 21x21 band from the fp16 Gram
tiles while unsharding. Shipped bytes per core: ~12.4MB out + 4.4MB in;
the kernel is DMA-bound at the cost model's 360GB/s with the serial-PE
matmul span (~34us) hidden underneath.
"""

import numpy as np

import concourse.mybir as mybir
import concourse.tile as tile
from concourse import bacc, bass
from concourse.bass_utils import run_bass_kernel_spmd

# Problem constants (hardcoded; kernel.py must be self-contained).
B, C, H, W = 4, 128, 128, 128
PAD = 10
WIN = 21  # correlation window side; WIN**2 = 441 output channels
N_CORES = 8
ROWS = H // 2  # 64 output rows per core
VROWS = ROWS + PAD  # 74 x2 rows a core's windows can touch
BANK = 512  # fp32 elements per PSUM bank

DI = 8  # block rows
NBI, NBJ = ROWS // DI, W // 8  # 8 block-rows, 16 8-wide block-cols
NR = DI + WIN - 1  # 28 window rows

# Per block-row bi: valid window rows = min(NR, VROWS - 8*bi), split evenly
# across the two PSUM banks.
VRR = [min(NR, VROWS - DI * bi) for bi in range(NBI)]  # 28,...,28,26,18
RSP = [v // 2 for v in VRR]  # per-bank rows: 14,...,14,13,9

KMIX = 2  # 8x16 blocks per block-row (PE/DMA balance knob)

# ---------------------------------------------------------------------------
# Unit table: one entry per PSUM-tile unit, in emission order.
#   pair unit: two mirrored 8x8 blocks (M=64 each) on the PE column halves
#   wide unit: one 8x16 block (M=128) on the full PE width
# Fields: kind, bi, per-grp pixel-col starts, x2 col read range, valid
# window width nsp, band col offsets, x1/gout offsets.
# ---------------------------------------------------------------------------


def _build_units(kmix):
    kml = [kmix] * NBI if isinstance(kmix, int) else list(kmix)
    units = []
    x1_off = 0
    g_off = 0
    for bi in range(NBI):
        km = kml[bi]
        r = RSP[bi]
        seq = []
        # mirrored edge pairs (bj, 15-bj) for bj = 0, 1
        for bj in range(2):
            seq.append(("pair", bj, NBJ - 1 - bj))
        # km wide blocks over bj 2,3 / 4,5 / ...
        for wset in range(km):
            seq.append(("wide", 2 + 2 * wset, None))
        # remaining interior as mirrored pairs
        lo, hi = 2 + 2 * km, NBJ - 3
        while lo < hi:
            seq.append(("pair", lo, hi))
            lo, hi = lo + 1, hi - 1
        for kind, a, b in seq:
            if kind == "pair":
                nsp = min(28, W + PAD - 8 * b, 8 * a + 18)
                u = dict(
                    kind=kind, bi=bi, r=r, nsp=nsp, ncol=r * nsp,
                    bjs=(a, b),
                    cst=(max(0, 8 * a - PAD), 8 * b - PAD),
                    soff=(max(0, PAD - 8 * a), 0),
                    x1_off=x1_off, g_off=g_off,
                )
                x1_off += 128
                g_off += 2 * u["ncol"]
            else:
                nsp = 36
                u = dict(
                    kind=kind, bi=bi, r=r, nsp=nsp, ncol=r * nsp,
                    c0=8 * a,  # pixel-col start (block covers 16 cols)
                    cst=8 * a - PAD,
                    x1_off=x1_off, g_off=g_off,
                )
                x1_off += 128
                g_off += 2 * u["ncol"]
            units.append(u)
    return units, x1_off, g_off


UNITS, X1_ELS, TOTAL_ELS = _build_units(KMIX)
NUNIT = len(UNITS)


def set_kmix(k):
    """Experimentation hook: rebuild the unit table for a different mix."""
    global KMIX, UNITS, X1_ELS, TOTAL_ELS, NUNIT
    KMIX = tuple(k) if not isinstance(k, int) else k
    UNITS, X1_ELS, TOTAL_ELS = _build_units(k)
    NUNIT = len(UNITS)
    _NC_CACHE.clear()

F32 = mybir.dt.float32
F16 = mybir.dt.float16

_NC_CACHE = {}

# Tunables (overridable via _build_nc kwargs for experiments).
GRAM_BUFS = 12
PSUM_BUFS = 4  # [128, 2, 512] tiles = 2 banks each


def _dve_cols(ncol):
    """DVE/ACT split of a 2-bank copy so both engines finish together."""
    return min(ncol, round(0.4444 * ncol + 20.5))


N_WARM = 12  # warm-up matmuls (PE p-state ramp) while inputs stream in
# Input chunks: (x1 el range, x2 row range). x2 rows for block-row bi are
# [8bi, 8bi+28); each chunk feeds one block-row ahead.
IN_CHUNKS = [
    ((0, 1024), (0, 28)),
    (None, (28, 36)),
    ((1024, 2048), (36, 44)),
    ((2048, 3072), (44, 52)),
    ((3072, 4096), (52, 60)),
    ((4096, 6144), (60, 68)),
    ((6144, 8192), (68, 74)),
]

# DMA batches: units per output DMA (UPB units per bi; 2 DMAs per bi).
QSCHED = None  # computed: split each bi's units into two halves


def _qsched():
    out = []
    for bi in range(NBI):
        n = sum(1 for u in UNITS if u["bi"] == bi)
        out.append(n - n // 2)
        out.append(n // 2)
    return out


def _build_nc(
    gram_bufs=None, psum_bufs=None, in_chunks=None,
    interleave=2, n_warm=None,
):
    gram_bufs = GRAM_BUFS if gram_bufs is None else gram_bufs
    psum_bufs = PSUM_BUFS if psum_bufs is None else psum_bufs
    in_chunks = IN_CHUNKS if in_chunks is None else in_chunks
    n_warm = N_WARM if n_warm is None else n_warm
    qsched = _qsched()
    assert sum(qsched) == NUNIT
    key = (
        KMIX, gram_bufs, psum_bufs,
        tuple((tuple(a) if a else None, tuple(b) if b else None) for a, b in in_chunks),
        interleave, n_warm,
    )
    if key in _NC_CACHE:
        return _NC_CACHE[key]
    nc = bacc.Bacc("TRN2", target_bir_lowering=False, debug=False, num_devices=N_CORES)
    # x1 arrives host-rearranged so each block's pixels are contiguous
    # (the matmul stationary operand AP must have a single free dimension).
    x1hd = nc.dram_tensor("x1h", [C, X1_ELS], F16, kind="ExternalInput")
    x2hd = nc.dram_tensor("x2h", [C, VROWS, W], F16, kind="ExternalInput")
    # Flat [partition, unit-major columns] fp16 Gram output.
    gout = nc.dram_tensor("gout", [128, TOTAL_ELS], F16, kind="ExternalOutput")

    # unit index -> (batch start unit, batch size)
    ustart = {}
    u0 = 0
    for qb in qsched:
        for p in range(u0, u0 + qb):
            ustart[p] = (u0, qb)
        u0 += qb

    with tile.TileContext(nc) as tc:
        with (
            tc.tile_pool(name="inp", bufs=1) as inp,
            tc.tile_pool(name="gram", bufs=gram_bufs) as gp,
            tc.tile_pool(name="psum", bufs=psum_bufs, space="PSUM") as pp,
        ):
            x1ht = inp.tile([C, X1_ELS], F16)
            x2ht = inp.tile([C, VROWS, W], F16)
            # Warm-up: ramp the PE p-state on an all-zero tile while the
            # first input chunks are still streaming in, and hoist the lazy
            # ACT table load (~1.3us) to t~0 with a 1-element dummy op.
            wt = inp.tile([128, 504], F16)
            nc.vector.memset(wt[:, :], 0.0)
            nc.scalar.copy(wt[:, 0:1], wt[:, 1:2])
            if n_warm:
                wps = pp.tile([128, 2, BANK], F32, tag="ps", name="wps")
                for _ in range(n_warm):
                    nc.tensor.matmul(
                        wps[0:64, 0, :504], wt[:, :64], wt[:, :],
                        start=True, stop=True,
                        tile_position=(0, 0), skip_group_check=True,
                    )
            for x1rng, x2rng in in_chunks:
                if x1rng is not None:
                    elo, ehi = x1rng
                    nc.sync.dma_start(x1ht[:, elo:ehi], x1hd[:, elo:ehi])
                if x2rng is not None:
                    rlo, rhi = x2rng
                    nc.sync.dma_start(x2ht[:, rlo:rhi, :], x2hd[:, rlo:rhi, :])

            def emit_half(u, h, ps):
                r = u["r"]
                rh = u["bi"] * DI + h * r
                ncol = u["ncol"]
                if u["kind"] == "pair":
                    for grp in range(2):
                        c0 = u["cst"][grp]
                        nc.tensor.matmul(
                            ps[64 * grp : 64 * grp + 64, h, :ncol],
                            x1ht[:, u["x1_off"] + 64 * grp : u["x1_off"] + 64 * grp + 64],
                            x2ht[:, rh : rh + r, c0 : c0 + u["nsp"]],
                            start=True, stop=True,
                            tile_position=(0, 64 * grp), skip_group_check=True,
                        )
                else:
                    nc.tensor.matmul(
                        ps[:, h, :ncol],
                        x1ht[:, u["x1_off"] : u["x1_off"] + 128],
                        x2ht[:, rh : rh + r, u["cst"] : u["cst"] + u["nsp"]],
                        start=True, stop=True,
                        tile_position=(0, 0), skip_group_check=True,
                    )

            tiles = {}
            for group in range(0, NUNIT, interleave):
                us = range(group, min(group + interleave, NUNIT))
                # h-major matmul order within the group: all h0 halves first,
                # so the PE is not head-of-line blocked on the x2 rows the h1
                # halves need while an input chunk is still in flight.
                for ui in us:
                    b0, qb = ustart[ui]
                    if ui == b0:
                        bels = sum(2 * UNITS[b0 + i]["ncol"] for i in range(qb))
                        tiles[b0] = gp.tile([128, bels], F16, tag="g", name="g")
                        tiles[("bels", b0)] = bels
                    ps = pp.tile([128, 2, BANK], F32, tag="ps", name="ps")
                    tiles[("ps", ui)] = ps
                    emit_half(UNITS[ui], 0, ps)
                for ui in us:
                    b0, qb = ustart[ui]
                    u = UNITS[ui]
                    emit_half(u, 1, tiles[("ps", ui)])
                    # copies fire right after this unit's h1 so the PSUM slot
                    # returns as early as possible
                    ps = tiles.pop(("ps", ui))
                    g = tiles[b0]
                    ncol = u["ncol"]
                    d = _dve_cols(ncol)
                    goff = u["g_off"] - UNITS[b0]["g_off"]
                    gt = g[:]
                    bels = tiles[("bels", b0)]
                    nc.vector.tensor_copy(
                        bass.AP(
                            tensor=gt.tensor, offset=gt.offset + goff,
                            ap=[[bels, 128], [ncol, 2], [1, d]],
                        ),
                        ps[:, :, :d],
                    )
                    if d < ncol:
                        nc.scalar.copy(
                            bass.AP(
                                tensor=gt.tensor, offset=gt.offset + goff + d,
                                ap=[[bels, 128], [ncol, 2], [1, ncol - d]],
                            ),
                            ps[:, :, d:ncol],
                        )
                    if ui == b0 + qb - 1:
                        off = UNITS[b0]["g_off"]
                        nc.sync.dma_start(
                            gout[:, off : off + bels], tiles.pop(b0)[:]
                        )
                        tiles.pop(("bels", b0))
    nc.compile()
    _NC_CACHE[key] = nc
    return nc


def _shard_inputs(x1, x2):
    """Per-core inputs: core k -> batch k//2, row-half k%2.

    Half-0 cores get their slab flipped vertically so the off-image halo
    rows sit past the end for every core (identical SPMD program); the host
    un-flips during extraction.
    """
    in_maps = []
    for k in range(N_CORES):
        b, half = k // 2, k % 2
        x1s = x1[b, :, 0:ROWS, :] if half == 0 else x1[b, :, ROWS:H, :]
        if half == 0:
            x1s = x1s[:, ::-1, :]
        x1s = x1s.astype(np.float16)
        x1r = np.empty((C, X1_ELS), dtype=np.float16)
        for u in UNITS:
            i0 = u["bi"] * DI
            if u["kind"] == "pair":
                for grp, bj in enumerate(u["bjs"]):
                    blkpx = x1s[:, i0 : i0 + DI, 8 * bj : 8 * bj + 8]
                    x1r[:, u["x1_off"] + 64 * grp : u["x1_off"] + 64 * grp + 64] = (
                        blkpx.reshape(C, 64)
                    )
            else:
                blkpx = x1s[:, i0 : i0 + DI, u["c0"] : u["c0"] + 16]
                x1r[:, u["x1_off"] : u["x1_off"] + 128] = blkpx.reshape(C, 128)
        if half == 0:
            # shipped row r = image row 73 - r (slab flipped)
            x2s = np.ascontiguousarray(x2[b, :, VROWS - 1 :: -1, :]).astype(np.float16)
        else:
            # shipped row q = image row 54 + q
            x2s = np.ascontiguousarray(x2[b, :, H - VROWS :, :]).astype(np.float16)
        in_maps.append({"x1h": x1r, "x2h": x2s})
    return in_maps


# Band-extraction index arrays (built once).
_IL = np.arange(DI).reshape(DI, 1, 1, 1)
_JL8 = np.arange(8).reshape(1, 8, 1, 1)
_JL16 = np.arange(16).reshape(1, 16, 1, 1)
_U = np.arange(WIN).reshape(1, 1, WIN, 1)
_V = np.arange(WIN).reshape(1, 1, 1, WIN)


def _extract_core_output(gout_np, flipped):
    """[128, TOTAL_ELS] fp16 Gram tiles -> [441, ROWS, W] fp32 output."""
    out = np.zeros((WIN, WIN, ROWS, W), dtype=np.float16)
    for u in UNITS:
        bi, r2, nsp, ncol = u["bi"], 2 * u["r"], u["nsp"], u["ncol"]
        cols = gout_np[:, u["g_off"] : u["g_off"] + 2 * ncol]
        i0 = bi * DI
        if u["kind"] == "pair":
            # partition p = 64*grp + il*8 + jl; free f = rr*nsp + ss
            g = cols.reshape(2, DI, 8, r2, nsp)
            for grp, bj in enumerate(u["bjs"]):
                gf = np.zeros((DI, 8, NR, 28), dtype=np.float16)
                gf[:, :, :r2, u["soff"][grp] : u["soff"][grp] + nsp] = g[grp]
                band = gf[_IL, _JL8, _IL + _U, _JL8 + _V]  # (DI, 8, WIN, WIN)
                out[:, :, i0 : i0 + DI, 8 * bj : 8 * bj + 8] = band.transpose(
                    2, 3, 0, 1
                )
        else:
            # partition p = il*16 + jl16; free f = rr*36 + ss
            g = cols.reshape(DI, 16, r2, nsp)
            gf = np.zeros((DI, 16, NR, 36), dtype=np.float16)
            gf[:, :, :r2, :] = g
            band = gf[_IL, _JL16, _IL + _U, _JL16 + _V]  # (DI, 16, WIN, WIN)
            out[:, :, i0 : i0 + DI, u["c0"] : u["c0"] + 16] = band.transpose(
                2, 3, 0, 1
            )
    if flipped:
        # device computed the vertically-flipped slab: u' = 20-u, i' = 63-i
        out = out[::-1, :, ::-1, :]
    return out.reshape(WIN * WIN, ROWS, W).astype(np.float32)


def kernel(x1: np.ndarray, x2: np.ndarray) -> np.ndarray:
    x1 = np.asarray(x1, dtype=np.float32)
    x2 = np.asarray(x2, dtype=np.float32)
    nc = _build_nc()
    in_maps = _shard_inputs(x1, x2)
    # Retry once: a freshly-claimed device occasionally reports a transient
    # NRT_EXEC_UNIT_UNRECOVERABLE on the first execution.
    try:
        res = run_bass_kernel_spmd(nc, in_maps, core_ids=list(range(N_CORES)))
    except Exception:
        import time as _time

        _time.sleep(5.0)
        res = run_bass_kernel_spmd(nc, in_maps, core_ids=list(range(N_CORES)))
    out = np.empty((B, WIN * WIN, H, W), dtype=np.float32)
    for k in range(N_CORES):
        b, half = k // 2, k % 2
        i0 = half * ROWS
        out[b, :, i0 : i0 + ROWS, :] = _extract_core_output(
            res.results[k]["gout"], flipped=(half == 0)
        )
    return out
